# revision 1
# baseline (speedup 1.0000x reference)
"""Real spherical harmonics Y_lm (l<=8) on 8 TRN2 NeuronCores.

Data-parallel over the 1M points. Per core: 125k points padded to
128*990; partition-major layout so each partition owns a contiguous
row range of the [N, 81] output -> output DMA is 128 large contiguous
runs per chunk.

All normalization constants are folded into the Legendre recurrences
(scaled P~ = ctil(l,m) * P_l^m), so each three-term recurrence is two
fused scalar_tensor_tensor ops and each output column is a single
tensor_tensor multiply with sin(m phi) / cos(m phi) from the ACT LUT.
"""

import math
import sys

sys.path.insert(0, "/opt/trn_rl_repo")

import numpy as np

import concourse.bass as bass
import concourse.mybir as mybir
from concourse.tile import TileContext
from concourse.bass_utils import run_bass_kernel_spmd

F32 = mybir.dt.float32
AF = mybir.ActivationFunctionType
OP = mybir.AluOpType

N_TOTAL = 1_000_000
NCORES = 8
PER = N_TOTAL // NCORES      # 125000 real points per core
P = 128                      # SBUF partitions
LPP = 990                    # points per partition (padded)
PADN = P * LPP               # 126720 padded points per core
FD = 198                     # free-dim chunk size
NCHUNK = LPP // FD           # 5
LMAX = 8
NCOL = (LMAX + 1) ** 2       # 81


# ACT Sin LUT domain is [-pi, pi]; we feed it t - pi with t = arg mod 2pi,
# which yields -sin(arg). The global -1 is folded into ctil(l,m) for m>=1
# (it cancels in every recurrence ratio, which are all within-m or
# diag-chain ctil ratios two m apart).
TWO_PI_LO = float(np.nextafter(np.float32(2 * math.pi), np.float32(0.0)))
PI_LO = float(np.nextafter(np.float32(math.pi), np.float32(0.0)))


def _ctil():
    """ctil(l,m) * P_l^m(x) * ang(m, phi) = output column, with the
    reference's 1/sqrt(2) for m=0 folded in. m>=1 entries are negated
    to absorb the -sin from the range-reduced LUT trig."""
    c = {}
    for l in range(LMAX + 1):
        c[(l, 0)] = math.sqrt((2 * l + 1) / (4 * math.pi))
        for m in range(1, l + 1):
            c[(l, m)] = -((-1.0) ** m) * math.sqrt(2.0) * math.sqrt(
                (2 * l + 1) / (4 * math.pi)
                * math.factorial(l - m) / math.factorial(l + m)
            )
    return c


def _lrec_ab(l, m, C):
    """P~(l,m) = a*x*P~(l-1,m) + b*P~(l-2,m)."""
    alpha = (2 * l - 1) / (l - m)
    beta = -(l + m - 1) / (l - m)
    a = alpha * C[(l, m)] / C[(l - 1, m)]
    b = beta * C[(l, m)] / C[(l - 2, m)]
    return a, b


def build_nc(lpp=LPP, fd=FD, fds=None):
    # fds: per-chunk free-dim sizes (sum == lpp). A smaller final chunk
    # shrinks the exposed tail DMA after the last compute finishes.
    if fds is None:
        fds = [fd] * (lpp // fd)
    assert sum(fds) == lpp
    padn = P * lpp
    C = _ctil()
    nc = bass.Bass()
    ct = nc.declare_dram_parameter("cos_theta", [padn], F32, isOutput=False)
    ph = nc.declare_dram_parameter("phi", [padn], F32, isOutput=False)
    out = nc.declare_dram_parameter("out", [padn * NCOL], F32, isOutput=True)

    ctv = ct[:].rearrange("(p f) -> p f", p=P)
    phv = ph[:].rearrange("(p f) -> p f", p=P)
    outv = out[:].rearrange("(p f) -> p f", p=P)

    with TileContext(nc) as tc:
        with (
            tc.tile_pool(name="res", bufs=1) as res_pool,
            tc.tile_pool(name="work", bufs=2) as work_pool,
            tc.tile_pool(name="obuf", bufs=2) as o_pool,
        ):
            xt = res_pool.tile([P, lpp], F32)
            pt = res_pool.tile([P, lpp], F32)
            # Sin needs AP biases (-pi and -pi/2); memset a const tile
            # inside the Tile context so deps are tracked (no barrier).
            cbias = res_pool.tile([P, 2], F32)
            nc.gpsimd.memset(cbias[:, 0:1], -PI_LO)
            nc.gpsimd.memset(cbias[:, 1:2], -PI_LO / 2)
            bias_negpi = cbias[:, 0:1]
            bias_neghalfpi = cbias[:, 1:2]

            off = 0
            for c, fd in enumerate(fds):
                sl = slice(off, off + fd)
                ocolbase = off * NCOL
                off += fd
                nc.sync.dma_start(out=xt[:, sl], in_=ctv[:, sl])
                nc.sync.dma_start(out=pt[:, sl], in_=phv[:, sl])
                x = xt[:, sl]
                f = pt[:, sl]

                w = work_pool.tile([P, fd * 38], F32)

                def W(i):
                    return w[:, i * fd:(i + 1) * fd]

                def WP(i):
                    # two adjacent fd slices as [P, 2, fd] (pair OUTER:
                    # the DVE then streams long stride runs, same as the
                    # unpaired ops, instead of a 2-element inner zigzag)
                    return w[:, i * fd:(i + 2) * fd].rearrange(
                        "p (k f) -> p k f", k=2
                    )

                def WF(i):
                    # two adjacent fd slices flat [P, 2*fd] (for all-
                    # contiguous paired ops, cheapest AP form)
                    return w[:, i * fd:(i + 2) * fd]

                x2, s, b, b2 = W(0), W(1), W(2), W(3)
                s2a, s2P = W(4), WP(4)          # s2 doubled
                TP = WP(6)                      # T pair
                twoC1a, twoC1b = W(8), W(9)
                twoC1F = WF(8)
                xxa, xxb, xxP = W(10), W(11), WP(10)
                DP = [WP(12 + 2 * k) for k in range(4)]   # diag pair ring
                DS = [W(12 + 2 * k) for k in range(4)]    # first slot of each
                uF = WF(20)
                SIN = [None] + [W(22 + 2 * (m - 1)) for m in range(1, 9)]
                COS = [None] + [W(23 + 2 * (m - 1)) for m in range(1, 9)]
                TRIGP = [None] + [WP(22 + 2 * (m - 1)) for m in range(1, 9)]
                TRIGF = [None] + [WF(22 + 2 * (m - 1)) for m in range(1, 9)]

                O = o_pool.tile([P, fd * NCOL], F32)
                O3 = O.rearrange("p (f c) -> p f c", c=NCOL)
                O3c = O.rearrange("p (f c) -> p c f", c=NCOL)

                def ocol(j):
                    return O3[:, :, j]

                def opair(j0, dm):
                    # columns j0 and j0+dm as [P, 2, fd] (pair outer)
                    return O3c[:, j0:j0 + dm + 1:dm, :]

                # ---- column (0,0) first: absorbs the WAR-vs-DMA wait on
                # this O slot in a single-dependency DVE op. O must only
                # ever be written by DVE (cross-engine writers would need
                # a second wait slot the TT ISA struct doesn't have).
                nc.vector.tensor_scalar(
                    ocol(0), x, 0.0, C[(0, 0)], OP.mult, OP.add
                )

                # ---- ACT: all single-source affine/transcendental work.
                # s = sqrt(1-x^2); doubled copies feed the paired DVE ops.
                nc.scalar.activation(x2, x, AF.Square)
                nc.scalar.activation(s, x2, AF.Sqrt, bias=1.0, scale=-1.0)
                nc.scalar.activation(s2a, x2, AF.Copy, scale=-1.0, bias=1.0)
                nc.scalar.activation(W(5), x2, AF.Copy, scale=-1.0, bias=1.0)
                nc.scalar.activation(xxa, x, AF.Copy)
                nc.scalar.activation(xxb, x, AF.Copy)
                # trig seeds: SIN[m]/COS[m] hold -sin/-cos(m phi) (the -1
                # lives in ctil). ACT Sin domain is [-pi,pi]:
                # SIN[1] = Sin(phi-pi) = -sin(phi); b = Sin(phi/2 - pi/2)
                # = -cos(phi/2); COS[1] = 1-2b^2 = -cos(phi);
                # twoC1 = 4b^2-2 = 2cos(phi). Chebyshev:
                # X'_m = twoC1*X'_{m-1} - X'_{m-2}, S'_0 = 0, C'_0 = -1.
                nc.scalar.activation(SIN[1], f, AF.Sin, bias=bias_negpi)
                nc.scalar.activation(
                    b, f, AF.Sin, scale=0.5, bias=bias_neghalfpi
                )
                nc.scalar.activation(b2, b, AF.Square)
                nc.scalar.activation(twoC1a, b2, AF.Copy, scale=4.0, bias=-2.0)
                nc.scalar.activation(twoC1b, b2, AF.Copy, scale=4.0, bias=-2.0)
                nc.scalar.activation(COS[1], b2, AF.Copy, scale=-2.0, bias=1.0)
                # diagonal seeds (doubled): P~(1,1) = -ctil(1,1)*s,
                # P~(2,2) = 3*ctil(2,2)*s^2
                nc.scalar.activation(DS[1], s, AF.Copy, scale=-C[(1, 1)])
                nc.scalar.activation(W(15), s, AF.Copy, scale=-C[(1, 1)])
                nc.scalar.activation(DS[2], s2a, AF.Copy, scale=3.0 * C[(2, 2)])
                nc.scalar.activation(W(17), s2a, AF.Copy, scale=3.0 * C[(2, 2)])

                # ---- DVE trig recurrence (paired sin|cos, flat APs) ----
                nc.vector.tensor_tensor(TRIGF[2], twoC1F, TRIGF[1], OP.mult)
                nc.vector.tensor_scalar(COS[2], COS[2], 1.0, None, OP.add)
                for m in range(3, 9):
                    nc.vector.tensor_tensor(uF, twoC1F, TRIGF[m - 1], OP.mult)
                    nc.vector.scalar_tensor_tensor(
                        TRIGF[m], TRIGF[m - 2], -1.0, uF, OP.mult, OP.add
                    )

                # ---- m = 0 chain: P~(l,0) is directly column l*l+l ----
                T0 = W(6)
                nc.vector.tensor_scalar(ocol(2), x, C[(1, 0)], None, OP.mult)
                a, bb = _lrec_ab(2, 0, C)
                nc.vector.scalar_tensor_tensor(T0, ocol(2), a, x, OP.mult, OP.mult)
                nc.vector.tensor_scalar(
                    ocol(6), T0, bb * C[(0, 0)], None, OP.add
                )
                for l in range(3, 9):
                    a, bb = _lrec_ab(l, 0, C)
                    nc.vector.scalar_tensor_tensor(
                        T0, ocol((l - 1) * l), a, x, OP.mult, OP.mult
                    )
                    nc.vector.scalar_tensor_tensor(
                        ocol(l * l + l), ocol((l - 2) * (l - 1)), bb, T0,
                        OP.mult, OP.add,
                    )

                # ---- m >= 1: columns satisfy the l-recurrence directly
                # (it is linear, the trig factor distributes), so all
                # work runs on +-m column PAIRS in one instruction. ----
                for m in range(1, 9):
                    if m >= 3:
                        Am = (2 * m - 1) * (2 * m - 3) * C[(m, m)] / C[(m - 2, m - 2)]
                        nc.vector.scalar_tensor_tensor(
                            DP[m & 3], DP[(m - 2) & 3], Am, s2P,
                            OP.mult, OP.mult,
                        )
                    jb = m * m + m
                    nc.vector.tensor_tensor(
                        opair(jb - m, 2 * m), DP[m & 3], TRIGP[m], OP.mult
                    )
                    if m <= 7:
                        Em = (2 * m + 1) * C[(m + 1, m)] / C[(m, m)]
                        j1 = (m + 1) * (m + 2)
                        nc.vector.scalar_tensor_tensor(
                            opair(j1 - m, 2 * m), opair(jb - m, 2 * m), Em,
                            xxP, OP.mult, OP.mult,
                        )
                        for l in range(m + 2, 9):
                            a, bb = _lrec_ab(l, m, C)
                            nc.vector.scalar_tensor_tensor(
                                TP, opair((l - 1) * l - m, 2 * m), a, xxP,
                                OP.mult, OP.mult,
                            )
                            nc.vector.scalar_tensor_tensor(
                                opair(l * l + l - m, 2 * m),
                                opair((l - 2) * (l - 1) - m, 2 * m), bb, TP,
                                OP.mult, OP.add,
                            )

                nc.sync.dma_start(
                    out=outv[:, ocolbase:ocolbase + fd * NCOL],
                    in_=O[:, :],
                )
    _legalize_waits(nc)
    return nc


_TPB_COMPUTE = (
    mybir.InstTensorTensor,
    mybir.InstTensorScalarPtr,
    mybir.InstActivation,
    mybir.InstTensorCopy,
    mybir.InstTensorReduce,
    mybir.InstMemset,
)


def _legalize_waits(nc):
    """TPB compute ISA structs encode a single sync-wait slot; Tile can
    emit 2+ waits on one instruction (walrus then fails with 'Too many
    sync wait commands'). Hoist all but one wait onto NoOps in front."""
    f = nc.m.functions[0]
    for b in f.blocks:
        insts = b.instructions
        idx = 0
        while idx < len(insts):
            i = insts[idx]
            si = i.sync_info
            if si is not None and len(si.on_wait) > 1:
                waits = list(si.on_wait)
                for wextra in waits[:-1]:
                    nop = mybir.InstEventSemaphore(
                        name=nc.get_next_instruction_name(), ins=[], outs=[]
                    )
                    nop.engine = i.engine
                    nop.sync_info = mybir.SyncInfo(
                        on_wait=[wextra], on_update=[]
                    )
                    nc.register_instruction(nop)
                    insts.insert(idx, nop)
                    idx += 1
                si.on_wait = [waits[-1]]
            idx += 1


_NC_CACHE = None


# Uneven chunks: clock-normalized DVE busy is identical to uniform
# (290.5 us, verified in-trace), but the smaller final chunk cuts the
# exposed tail DMA from 8.2 MB (23 us) to 5.6 MB (15.5 us).
# [216x4, 126] no longer fits SBUF; 214 is the limit.
FDS = [214, 214, 214, 214, 134]


def _get_nc():
    global _NC_CACHE
    if _NC_CACHE is None:
        _NC_CACHE = build_nc(fds=FDS)
    return _NC_CACHE


# NOTE: identical NEFFs measure either ~324 us or ~384 us depending on
# which physical cores the process lands on (DVE/ACT at 0.96 vs 0.8
# GHz — visible as an exact 1.2x scale on engine-busy in the trace).
# In-process warm-up bursts do not change the state; it is placement/
# machine-side, so no kernel-side mitigation exists.
def _run(cos_theta, phi, trace=False, **kw):
    cos_theta = np.ascontiguousarray(np.asarray(cos_theta), dtype=np.float32)
    phi = np.ascontiguousarray(np.asarray(phi), dtype=np.float32)
    assert cos_theta.shape == (N_TOTAL,) and phi.shape == (N_TOTAL,)
    in_maps = []
    for i in range(NCORES):
        c = np.zeros(PADN, np.float32)
        p_ = np.zeros(PADN, np.float32)
        c[:PER] = cos_theta[i * PER:(i + 1) * PER]
        p_[:PER] = phi[i * PER:(i + 1) * PER]
        in_maps.append({"cos_theta": c, "phi": p_})
    res = run_bass_kernel_spmd(
        _get_nc(), in_maps, core_ids=list(range(NCORES)), trace=trace, **kw
    )
    outs = [
        np.asarray(r["out"]).reshape(PADN, NCOL)[:PER] for r in res.results
    ]
    return np.concatenate(outs, axis=0), res


def kernel(cos_theta, phi):
    out, _ = _run(cos_theta, phi)
    return out



# revision 6
# speedup vs baseline: 1.4031x; 1.4031x over previous
"""Real spherical harmonics Y_lm (l<=8) on 8 TRN2 NeuronCores.

Data-parallel over the 1M points; per core 125k points padded to
128*977. Output is written COLUMN-MAJOR per chunk ([P, col, f]) in
fp16 — every engine op then streams contiguous fd-long runs (no
stride-81 element zigzags), and output DMA bytes halve. The host
unshard transposes back to [N, 81] f32 (rel-err budget: fp16 state
in the l-recurrences costs ~1e-3 vs the 2e-2 gate).

Engine split: ACT computes all single-source seeds (trig seeds,
sqrt, doubled pair operands); Pool (gpsimd) runs the m=1..3 column
chains; DVE runs the f32 trig Chebyshev recurrence, diagonal chain,
column seed multiplies, m=0 chain, and the m=4..7 chains.
"""

import math
import sys

sys.path.insert(0, "/opt/trn_rl_repo")

import numpy as np

import concourse.bass as bass
import concourse.mybir as mybir
from concourse.tile import TileContext
from concourse.bass_utils import run_bass_kernel_spmd

F32 = mybir.dt.float32
F16 = mybir.dt.float16
AF = mybir.ActivationFunctionType
OP = mybir.AluOpType

N_TOTAL = 1_000_000
NCORES = 8
PER = N_TOTAL // NCORES      # 125000 real points per core
P = 128                      # SBUF partitions
LPP = 977                    # points per partition (padded, 125056)
PADN = P * LPP
LMAX = 8
NCOL = (LMAX + 1) ** 2       # 81
FDS = [300, 300, 300, 77]    # free-dim chunk sizes (sum == LPP)

# ACT Sin LUT domain is [-pi, pi]; we feed t - pi, yielding -sin(t).
# The global -1 is folded into ctil(l,m) for m>=1.
TWO_PI_LO = float(np.nextafter(np.float32(2 * math.pi), np.float32(0.0)))
PI_LO = float(np.nextafter(np.float32(math.pi), np.float32(0.0)))


def _ctil():
    """ctil(l,m) * P_l^m(x) * ang(m, phi) = output column, with the
    reference's 1/sqrt(2) for m=0 folded in. m>=1 entries are negated
    to absorb the -sin from the range-reduced LUT trig."""
    c = {}
    for l in range(LMAX + 1):
        c[(l, 0)] = math.sqrt((2 * l + 1) / (4 * math.pi))
        for m in range(1, l + 1):
            c[(l, m)] = -((-1.0) ** m) * math.sqrt(2.0) * math.sqrt(
                (2 * l + 1) / (4 * math.pi)
                * math.factorial(l - m) / math.factorial(l + m)
            )
    return c


def _lrec_ab(l, m, C):
    """P~(l,m) = a*x*P~(l-1,m) + b*P~(l-2,m)."""
    alpha = (2 * l - 1) / (l - m)
    beta = -(l + m - 1) / (l - m)
    a = alpha * C[(l, m)] / C[(l - 1, m)]
    b = beta * C[(l, m)] / C[(l - 2, m)]
    return a, b


# Pool's ISA has no TensorScalarPtr (STT); it runs the TT-shaped work
# (trig Chebyshev, column-seed multiplies) while DVE runs all STT.
POOL_CHAINS = ()


def build_nc(fds=None):
    if fds is None:
        fds = FDS
    lpp = sum(fds)
    assert lpp == LPP
    C = _ctil()
    nc = bass.Bass()
    ct = nc.declare_dram_parameter("cos_theta", [PADN], F32, isOutput=False)
    ph = nc.declare_dram_parameter("phi", [PADN], F32, isOutput=False)
    out = nc.declare_dram_parameter("out", [PADN * NCOL], F16, isOutput=True)

    ctv = ct[:].rearrange("(p f) -> p f", p=P)
    phv = ph[:].rearrange("(p f) -> p f", p=P)
    outv = out[:].rearrange("(p f) -> p f", p=P)

    V = None  # set below (nc.vector)

    with TileContext(nc) as tc:
        with (
            tc.tile_pool(name="res", bufs=1) as res_pool,
            tc.tile_pool(name="work", bufs=2) as work_pool,
            tc.tile_pool(name="obuf", bufs=2) as o_pool,
        ):
            V = nc.vector
            G = nc.gpsimd
            A = nc.scalar

            xt = res_pool.tile([P, lpp], F32)
            pt = res_pool.tile([P, lpp], F32)
            cbias = res_pool.tile([P, 2], F32)
            G.memset(cbias[:, 0:1], -PI_LO)
            G.memset(cbias[:, 1:2], -PI_LO / 2)
            bias_negpi = cbias[:, 0:1]
            bias_neghalfpi = cbias[:, 1:2]

            off = 0
            for fd in fds:
                sl = slice(off, off + fd)
                obase = off * NCOL
                off += fd
                nc.sync.dma_start(out=xt[:, sl], in_=ctv[:, sl])
                nc.sync.dma_start(out=pt[:, sl], in_=phv[:, sl])
                x = xt[:, sl]
                f = pt[:, sl]

                # f32 work: 34 fd-slices.
                #  0 x2 | 1 s | 2,3 s2 pair | 4..11 diag ring (4 pairs)
                #  12,13 twoC1 pair | 14,15 trig temp u | 16..31 trig
                #  (SIN|COS pairs, m at 16+2(m-1)) | 32 b | 33 b2
                w = work_pool.tile([P, fd * 34], F32)
                # fp16 work: 0 xx | 1 xx dup | 2,3 TP(dve) | 4,5 TP(pool)
                #  6 T0 (m=0 chain temp)
                w6 = work_pool.tile([P, fd * 7], F16)

                def W(i):
                    return w[:, i * fd:(i + 1) * fd]

                def WF(i):
                    return w[:, i * fd:(i + 2) * fd]

                def WP(i):
                    return w[:, i * fd:(i + 2) * fd].rearrange(
                        "p (k f) -> p k f", k=2
                    )

                def H(i):
                    return w6[:, i * fd:(i + 1) * fd]

                def HP(i):
                    return w6[:, i * fd:(i + 2) * fd].rearrange(
                        "p (k f) -> p k f", k=2
                    )

                XX = H(0)
                XXP = HP(0)
                TPD = HP(2)   # DVE chains' temp pair
                TPP = HP(4)   # Pool chains' temp pair
                T0 = H(6)

                def DP(m):
                    return WP(4 + 2 * (m & 3))

                def TRIGF(m):
                    return WF(16 + 2 * (m - 1))

                def TRIGP(m):
                    return WP(16 + 2 * (m - 1))

                O = o_pool.tile([P, fd * NCOL], F16)
                O3c = O.rearrange("p (c f) -> p c f", c=NCOL)

                def ocol(j):
                    return O[:, j * fd:(j + 1) * fd]

                def opair(j0, dm):
                    return O3c[:, j0:j0 + dm + 1:dm, :]

                # ---- ACT: all single-source seeds ----
                A.activation(W(0), x, AF.Square)                      # x^2
                A.activation(W(1), W(0), AF.Sqrt, bias=1.0, scale=-1.0)  # s
                A.activation(W(2), W(0), AF.Copy, scale=-1.0, bias=1.0)  # s2
                A.activation(W(3), W(0), AF.Copy, scale=-1.0, bias=1.0)
                c11, c22 = C[(1, 1)], C[(2, 2)]
                A.activation(W(6), W(1), AF.Copy, scale=-c11)  # D1 pair
                A.activation(W(7), W(1), AF.Copy, scale=-c11)
                A.activation(W(8), W(0), AF.Copy, scale=-3 * c22, bias=3 * c22)
                A.activation(W(9), W(0), AF.Copy, scale=-3 * c22, bias=3 * c22)
                A.activation(H(0), x, AF.Copy)                 # xx fp16 pair
                A.activation(H(1), x, AF.Copy)
                A.activation(W(16), f, AF.Sin, bias=bias_negpi)   # SIN1
                A.activation(W(32), f, AF.Sin, scale=0.5, bias=bias_neghalfpi)
                A.activation(W(33), W(32), AF.Square)             # b^2
                A.activation(W(12), W(33), AF.Copy, scale=4.0, bias=-2.0)
                A.activation(W(13), W(33), AF.Copy, scale=4.0, bias=-2.0)
                A.activation(W(17), W(33), AF.Copy, scale=-2.0, bias=1.0)

                # ---- Pool: f32 trig Chebyshev on SIN|COS pairs ----
                G.tensor_tensor(TRIGF(2), WF(12), TRIGF(1), OP.mult)
                G.tensor_scalar(W(19), W(19), 1.0, None, OP.add)  # COS2 += 1
                for m in range(3, 9):
                    G.tensor_tensor(WF(14), WF(12), TRIGF(m - 1), OP.mult)
                    G.tensor_tensor(TRIGF(m), WF(14), TRIGF(m - 2), OP.subtract)

                # ---- m = 0 chain (fp16 state lives in the columns) ----
                G.memset(ocol(0), C[(0, 0)])
                V.tensor_scalar(ocol(2), x, C[(1, 0)], None, OP.mult)
                a, bb = _lrec_ab(2, 0, C)
                V.scalar_tensor_tensor(T0, ocol(2), a, XX, OP.mult, OP.mult)
                V.tensor_scalar(ocol(6), T0, bb * C[(0, 0)], None, OP.add)
                for l in range(3, 9):
                    a, bb = _lrec_ab(l, 0, C)
                    V.scalar_tensor_tensor(
                        T0, ocol((l - 1) * l), a, XX, OP.mult, OP.mult
                    )
                    V.scalar_tensor_tensor(
                        ocol(l * l + l), ocol((l - 2) * (l - 1)), bb, T0,
                        OP.mult, OP.add,
                    )

                # ---- per-m: diag (DVE STT, 4-pair ring — the colmult
                # read must precede the m+4 overwrite, hence the
                # interleaved order), column seed (Pool TT), then the
                # fp16 first-l + l-recurrence (DVE STT) ----
                for m in range(1, 9):
                    if m >= 3:
                        Am = (2 * m - 1) * (2 * m - 3) * C[(m, m)] / C[(m - 2, m - 2)]
                        V.scalar_tensor_tensor(
                            DP(m), DP(m - 2), Am, WP(2), OP.mult, OP.mult
                        )
                    G.tensor_tensor(opair(m * m, 2 * m), DP(m), TRIGP(m), OP.mult)
                    if m > 7:
                        continue
                    E, TP = (G, TPP) if m in POOL_CHAINS else (V, TPD)
                    Em = (2 * m + 1) * C[(m + 1, m)] / C[(m, m)]
                    jb = m * m + m
                    j1 = (m + 1) * (m + 2)
                    E.scalar_tensor_tensor(
                        opair(j1 - m, 2 * m), opair(jb - m, 2 * m), Em,
                        XXP, OP.mult, OP.mult,
                    )
                    for l in range(m + 2, 9):
                        a, bb = _lrec_ab(l, m, C)
                        E.scalar_tensor_tensor(
                            TP, opair((l - 1) * l - m, 2 * m), a, XXP,
                            OP.mult, OP.mult,
                        )
                        E.scalar_tensor_tensor(
                            opair(l * l + l - m, 2 * m),
                            opair((l - 2) * (l - 1) - m, 2 * m), bb, TP,
                            OP.mult, OP.add,
                        )

                nc.sync.dma_start(
                    out=outv[:, obase:obase + fd * NCOL], in_=O[:, :]
                )
    _legalize_waits(nc)
    return nc


def _legalize_waits(nc):
    """TPB compute ISA structs encode a single sync-wait slot; Tile can
    emit 2+ waits on one instruction (walrus then fails with 'Too many
    sync wait commands'). Hoist all but one wait onto NoOps in front."""
    f = nc.m.functions[0]
    for b in f.blocks:
        insts = b.instructions
        idx = 0
        while idx < len(insts):
            i = insts[idx]
            si = i.sync_info
            if si is not None and len(si.on_wait) > 1:
                waits = list(si.on_wait)
                for wextra in waits[:-1]:
                    nop = mybir.InstEventSemaphore(
                        name=nc.get_next_instruction_name(), ins=[], outs=[]
                    )
                    nop.engine = i.engine
                    nop.sync_info = mybir.SyncInfo(
                        on_wait=[wextra], on_update=[]
                    )
                    nc.register_instruction(nop)
                    insts.insert(idx, nop)
                    idx += 1
                si.on_wait = [waits[-1]]
            idx += 1


_NC_CACHE = None


def _get_nc():
    global _NC_CACHE
    if _NC_CACHE is None:
        _NC_CACHE = build_nc()
    return _NC_CACHE


def _run(cos_theta, phi, trace=False, **kw):
    cos_theta = np.ascontiguousarray(np.asarray(cos_theta), dtype=np.float32)
    phi = np.ascontiguousarray(np.asarray(phi), dtype=np.float32)
    assert cos_theta.shape == (N_TOTAL,) and phi.shape == (N_TOTAL,)
    in_maps = []
    for i in range(NCORES):
        c = np.zeros(PADN, np.float32)
        p_ = np.zeros(PADN, np.float32)
        c[:PER] = cos_theta[i * PER:(i + 1) * PER]
        p_[:PER] = phi[i * PER:(i + 1) * PER]
        in_maps.append({"cos_theta": c, "phi": p_})
    res = run_bass_kernel_spmd(
        _get_nc(), in_maps, core_ids=list(range(NCORES)), trace=trace, **kw
    )
    outs = []
    for r in res.results:
        o = np.asarray(r["out"]).reshape(P, LPP * NCOL)  # fp16, col-major
        full = np.empty((P, LPP, NCOL), np.float32)
        offp = 0
        for fd in FDS:
            blk = o[:, offp * NCOL:(offp + fd) * NCOL].reshape(P, NCOL, fd)
            full[:, offp:offp + fd, :] = blk.transpose(0, 2, 1)
            offp += fd
        outs.append(full.reshape(PADN, NCOL)[:PER])
    return np.concatenate(outs, axis=0), res


def kernel(cos_theta, phi):
    out, _ = _run(cos_theta, phi)
    return out


# revision 7
# speedup vs baseline: 1.4640x; 1.0434x over previous
"""Real spherical harmonics Y_lm (l<=8) on 8 TRN2 NeuronCores.

Data-parallel over the 1M points; per core 125k points padded to
128*977. Output is written in fp16 with a PERMUTED column order:
within each chunk the SBUF/DRAM layout is [P, slot, f] where the 81
slots put each chain's (-m,+m) column pair ADJACENT (m=0 columns
first, then per-m (sin,cos) pairs by l). Every engine op then reads/
writes flat contiguous [P, fd] / [P, 2*fd] runs -- strided fp16 pair
access measured ~3x slower on DVE. The host unshard applies the
inverse permutation and converts to f32 (fp16 state in the
l-recurrences costs ~1.3e-3 rel err vs the 2e-2 gate).

Engine split (measured rates): DVE runs all STT work (m=0 chain,
diagonal chain, first-l, l-recurrences) plus the trig fixups; Pool
runs TT-shaped f32 work (trig Chebyshev, column-seed multiplies);
ACT computes all single-source seeds (trig seeds first -- they gate
Pool's chain).
"""

import math
import sys

sys.path.insert(0, "/opt/trn_rl_repo")

import numpy as np

import concourse.bass as bass
import concourse.mybir as mybir
from concourse.tile import TileContext
from concourse.bass_utils import run_bass_kernel_spmd

F32 = mybir.dt.float32
F16 = mybir.dt.float16
AF = mybir.ActivationFunctionType
OP = mybir.AluOpType

N_TOTAL = 1_000_000
NCORES = 8
PER = N_TOTAL // NCORES      # 125000 real points per core
P = 128                      # SBUF partitions
LPP = 977                    # points per partition (padded: 125056)
PADN = P * LPP
LMAX = 8
NCOL = (LMAX + 1) ** 2       # 81
FDS = [300, 300, 300, 77]    # free-dim chunk sizes (sum == LPP)

# ACT Sin LUT domain is [-pi, pi]; we feed t - pi, yielding -sin(t).
# The global -1 is folded into ctil(l,m) for m>=1.
PI_LO = float(np.nextafter(np.float32(math.pi), np.float32(0.0)))

# --- output slot permutation: m=0 columns at slots 0..8, then each
# m-chain's (sin|cos) pairs adjacent, ordered by l ---
ORDER = [l * l + l for l in range(LMAX + 1)]
PAIRBASE = {}
_slot = LMAX + 1
for _m in range(1, LMAX + 1):
    for _l in range(_m, LMAX + 1):
        PAIRBASE[(_l, _m)] = _slot
        ORDER.append(_l * _l + _l - _m)
        ORDER.append(_l * _l + _l + _m)
        _slot += 2
POSARR = np.empty(NCOL, np.int64)
for _i, _j in enumerate(ORDER):
    POSARR[_j] = _i


def _ctil():
    """ctil(l,m) * P_l^m(x) * ang(m, phi) = output column, with the
    reference's 1/sqrt(2) for m=0 folded in. m>=1 entries are negated
    to absorb the -sin from the range-reduced LUT trig."""
    c = {}
    for l in range(LMAX + 1):
        c[(l, 0)] = math.sqrt((2 * l + 1) / (4 * math.pi))
        for m in range(1, l + 1):
            c[(l, m)] = -((-1.0) ** m) * math.sqrt(2.0) * math.sqrt(
                (2 * l + 1) / (4 * math.pi)
                * math.factorial(l - m) / math.factorial(l + m)
            )
    return c


def _lrec_ab(l, m, C):
    """P~(l,m) = a*x*P~(l-1,m) + b*P~(l-2,m)."""
    alpha = (2 * l - 1) / (l - m)
    beta = -(l + m - 1) / (l - m)
    a = alpha * C[(l, m)] / C[(l - 1, m)]
    b = beta * C[(l, m)] / C[(l - 2, m)]
    return a, b


def build_nc(fds=None):
    if fds is None:
        fds = FDS
    lpp = sum(fds)
    assert lpp == LPP
    C = _ctil()
    nc = bass.Bass()
    ct = nc.declare_dram_parameter("cos_theta", [PADN], F32, isOutput=False)
    ph = nc.declare_dram_parameter("phi", [PADN], F32, isOutput=False)
    out = nc.declare_dram_parameter("out", [PADN * NCOL], F16, isOutput=True)

    ctv = ct[:].rearrange("(p f) -> p f", p=P)
    phv = ph[:].rearrange("(p f) -> p f", p=P)
    outv = out[:].rearrange("(p f) -> p f", p=P)

    with TileContext(nc) as tc:
        with (
            tc.tile_pool(name="res", bufs=1) as res_pool,
            tc.tile_pool(name="work", bufs=2) as work_pool,
            tc.tile_pool(name="obuf", bufs=2) as o_pool,
        ):
            V = nc.vector
            G = nc.gpsimd
            A = nc.scalar

            xt = res_pool.tile([P, lpp], F32)
            pt = res_pool.tile([P, lpp], F32)
            cbias = res_pool.tile([P, 2], F32)
            G.memset(cbias[:, 0:1], -PI_LO)
            G.memset(cbias[:, 1:2], -PI_LO / 2)
            bias_negpi = cbias[:, 0:1]
            bias_neghalfpi = cbias[:, 1:2]

            off = 0
            for fd in fds:
                sl = slice(off, off + fd)
                obase = off * NCOL
                off += fd
                nc.sync.dma_start(out=xt[:, sl], in_=ctv[:, sl])
                nc.sync.dma_start(out=pt[:, sl], in_=phv[:, sl])
                x = xt[:, sl]
                f = pt[:, sl]

                # f32 work: 34 fd-slices, all pairs flat/adjacent.
                #  0 x2 | 1 s | 2,3 s2 pair | 4..11 diag ring (4 pairs)
                #  12,13 twoC1 pair | 14,15 trig temp u | 16..31 trig
                #  (SIN|COS pairs, m at 16+2(m-1)) | 32 b | 33 b2
                w = work_pool.tile([P, fd * 34], F32)
                # fp16 work: 0,1 xx pair | 2,3 T pair | 4 T0 (m=0)
                w6 = work_pool.tile([P, fd * 5], F16)

                def W(i):
                    return w[:, i * fd:(i + 1) * fd]

                def WF(i):
                    return w[:, i * fd:(i + 2) * fd]

                def H(i):
                    return w6[:, i * fd:(i + 1) * fd]

                def HF(i):
                    return w6[:, i * fd:(i + 2) * fd]

                XX = H(0)
                XXP = HF(0)
                TP = HF(2)
                T0 = H(4)

                def DPf(m):
                    return WF(4 + 2 * (m & 3))

                def TRIGF(m):
                    return WF(16 + 2 * (m - 1))

                O = o_pool.tile([P, fd * NCOL], F16)

                def om0(l):
                    return O[:, l * fd:(l + 1) * fd]

                def opr(l, m):
                    b0 = PAIRBASE[(l, m)]
                    return O[:, b0 * fd:(b0 + 2) * fd]

                # ---- ACT seeds; trig group first (gates Pool) ----
                A.activation(W(16), f, AF.Sin, bias=bias_negpi)   # SIN1
                A.activation(W(32), f, AF.Sin, scale=0.5, bias=bias_neghalfpi)
                A.activation(W(33), W(32), AF.Square)             # b^2
                A.activation(W(12), W(33), AF.Copy, scale=4.0, bias=-2.0)
                A.activation(W(13), W(33), AF.Copy, scale=4.0, bias=-2.0)
                A.activation(W(17), W(33), AF.Copy, scale=-2.0, bias=1.0)
                A.activation(W(0), x, AF.Square)                      # x^2
                A.activation(W(1), W(0), AF.Sqrt, bias=1.0, scale=-1.0)  # s
                A.activation(W(2), W(0), AF.Copy, scale=-1.0, bias=1.0)  # s2
                A.activation(W(3), W(0), AF.Copy, scale=-1.0, bias=1.0)
                c11, c22 = C[(1, 1)], C[(2, 2)]
                A.activation(W(6), W(1), AF.Copy, scale=-c11)  # D1 pair
                A.activation(W(7), W(1), AF.Copy, scale=-c11)
                A.activation(W(8), W(0), AF.Copy, scale=-3 * c22, bias=3 * c22)
                A.activation(W(9), W(0), AF.Copy, scale=-3 * c22, bias=3 * c22)
                A.activation(H(0), x, AF.Copy)                 # xx fp16 pair
                A.activation(H(1), x, AF.Copy)

                # ---- Pool: f32 trig Chebyshev on SIN|COS pairs;
                # the m=2 cos fixup (+1) runs on DVE (Pool TS is slow)
                G.tensor_tensor(TRIGF(2), WF(12), TRIGF(1), OP.mult)
                V.tensor_scalar(W(19), W(19), 1.0, None, OP.add)  # COS2 += 1
                for m in range(3, 9):
                    G.tensor_tensor(WF(14), WF(12), TRIGF(m - 1), OP.mult)
                    G.tensor_tensor(TRIGF(m), WF(14), TRIGF(m - 2), OP.subtract)

                # ---- m = 0 chain (fp16 state lives in the columns) ----
                G.memset(om0(0), C[(0, 0)])
                V.tensor_scalar(om0(1), x, C[(1, 0)], None, OP.mult)
                a, bb = _lrec_ab(2, 0, C)
                V.scalar_tensor_tensor(T0, om0(1), a, XX, OP.mult, OP.mult)
                V.tensor_scalar(om0(2), T0, bb * C[(0, 0)], None, OP.add)
                for l in range(3, 9):
                    a, bb = _lrec_ab(l, 0, C)
                    V.scalar_tensor_tensor(T0, om0(l - 1), a, XX, OP.mult, OP.mult)
                    V.scalar_tensor_tensor(
                        om0(l), om0(l - 2), bb, T0, OP.mult, OP.add
                    )

                # ---- per-m: diag (DVE STT, 4-pair ring -- colmult must
                # consume slot m before diag m+4 overwrites it, hence
                # interleaved), column seed (Pool TT), fp16 chains (DVE)
                for m in range(1, 9):
                    if m >= 3:
                        Am = (2 * m - 1) * (2 * m - 3) * C[(m, m)] / C[(m - 2, m - 2)]
                        V.scalar_tensor_tensor(
                            DPf(m), DPf(m - 2), Am, WF(2), OP.mult, OP.mult
                        )
                    G.tensor_tensor(opr(m, m), DPf(m), TRIGF(m), OP.mult)
                    if m > 7:
                        continue
                    Em = (2 * m + 1) * C[(m + 1, m)] / C[(m, m)]
                    V.scalar_tensor_tensor(
                        opr(m + 1, m), opr(m, m), Em, XXP, OP.mult, OP.mult
                    )
                    for l in range(m + 2, 9):
                        a, bb = _lrec_ab(l, m, C)
                        V.scalar_tensor_tensor(
                            TP, opr(l - 1, m), a, XXP, OP.mult, OP.mult
                        )
                        V.scalar_tensor_tensor(
                            opr(l, m), opr(l - 2, m), bb, TP, OP.mult, OP.add
                        )

                nc.sync.dma_start(
                    out=outv[:, obase:obase + fd * NCOL], in_=O[:, :]
                )
    _legalize_waits(nc)
    return nc


def _legalize_waits(nc):
    """TPB compute ISA structs encode a single sync-wait slot; Tile can
    emit 2+ waits on one instruction (walrus then fails with 'Too many
    sync wait commands'). Hoist all but one wait onto NoOps in front."""
    f = nc.m.functions[0]
    for b in f.blocks:
        insts = b.instructions
        idx = 0
        while idx < len(insts):
            i = insts[idx]
            si = i.sync_info
            if si is not None and len(si.on_wait) > 1:
                waits = list(si.on_wait)
                for wextra in waits[:-1]:
                    nop = mybir.InstEventSemaphore(
                        name=nc.get_next_instruction_name(), ins=[], outs=[]
                    )
                    nop.engine = i.engine
                    nop.sync_info = mybir.SyncInfo(
                        on_wait=[wextra], on_update=[]
                    )
                    nc.register_instruction(nop)
                    insts.insert(idx, nop)
                    idx += 1
                si.on_wait = [waits[-1]]
            idx += 1


_NC_CACHE = None


def _get_nc():
    global _NC_CACHE
    if _NC_CACHE is None:
        _NC_CACHE = build_nc()
    return _NC_CACHE


def _run(cos_theta, phi, trace=False, **kw):
    cos_theta = np.ascontiguousarray(np.asarray(cos_theta), dtype=np.float32)
    phi = np.ascontiguousarray(np.asarray(phi), dtype=np.float32)
    assert cos_theta.shape == (N_TOTAL,) and phi.shape == (N_TOTAL,)
    in_maps = []
    for i in range(NCORES):
        c = np.zeros(PADN, np.float32)
        p_ = np.zeros(PADN, np.float32)
        c[:PER] = cos_theta[i * PER:(i + 1) * PER]
        p_[:PER] = phi[i * PER:(i + 1) * PER]
        in_maps.append({"cos_theta": c, "phi": p_})
    res = run_bass_kernel_spmd(
        _get_nc(), in_maps, core_ids=list(range(NCORES)), trace=trace, **kw
    )
    outs = []
    for r in res.results:
        o = np.asarray(r["out"]).reshape(P, LPP * NCOL)  # fp16, slot-major
        full = np.empty((P, LPP, NCOL), np.float32)
        offp = 0
        for fd in FDS:
            blk = o[:, offp * NCOL:(offp + fd) * NCOL].reshape(P, NCOL, fd)
            full[:, offp:offp + fd, :] = blk[:, POSARR, :].transpose(0, 2, 1)
            offp += fd
        outs.append(full.reshape(PADN, NCOL)[:PER])
    return np.concatenate(outs, axis=0), res


def kernel(cos_theta, phi):
    out, _ = _run(cos_theta, phi)
    return out


# revision 9
# speedup vs baseline: 1.7293x; 1.1812x over previous
"""Real spherical harmonics Y_lm (l<=8) on 8 TRN2 NeuronCores.

Data-parallel over the 1M points; per core 125k points padded to
128*977. Output is written in fp16 with a PERMUTED column order:
within each chunk the SBUF/DRAM layout is [P, slot, f] where the 81
slots put each chain's (-m,+m) column pair ADJACENT (m=0 columns
first, then per-m (sin,cos) pairs by l). Every engine op then reads/
writes flat contiguous [P, fd] / [P, 2*fd] runs -- strided fp16 pair
access measured ~3x slower on DVE. The host unshard applies the
inverse permutation and converts to f32 (fp16 state in the
l-recurrences costs ~1.3e-3 rel err vs the 2e-2 gate).

Engine split (measured rates): DVE runs all STT work (m=0 chain,
diagonal chain, first-l, l-recurrences) plus the trig fixups; Pool
runs TT-shaped f32 work (trig Chebyshev, column-seed multiplies);
ACT computes all single-source seeds (trig seeds first -- they gate
Pool's chain).
"""

import math
import sys

sys.path.insert(0, "/opt/trn_rl_repo")

import numpy as np

import concourse.bass as bass
import concourse.mybir as mybir
from concourse.tile import TileContext
from concourse.bass_utils import run_bass_kernel_spmd

F32 = mybir.dt.float32
F16 = mybir.dt.float16
AF = mybir.ActivationFunctionType
OP = mybir.AluOpType

N_TOTAL = 1_000_000
NCORES = 8
PER = N_TOTAL // NCORES      # 125000 real points per core
P = 128                      # SBUF partitions
LPP = 977                    # points per partition (padded: 125056)
PADN = P * LPP
LMAX = 8
NCOL = (LMAX + 1) ** 2       # 81
FDS = [300, 300, 300, 77]    # free-dim chunk sizes (sum == LPP)

# ACT Sin LUT domain is [-pi, pi]; we feed t - pi, yielding -sin(t).
# The global -1 is folded into ctil(l,m) for m>=1.
PI_LO = float(np.nextafter(np.float32(math.pi), np.float32(0.0)))

# --- output slot permutation: m=0 columns at slots 0..8, then each
# m-chain's (sin|cos) pairs adjacent, ordered by l ---
ORDER = [l * l + l for l in range(LMAX + 1)]
PAIRBASE = {}
_slot = LMAX + 1
for _m in range(1, LMAX + 1):
    for _l in range(_m, LMAX + 1):
        PAIRBASE[(_l, _m)] = _slot
        ORDER.append(_l * _l + _l - _m)
        ORDER.append(_l * _l + _l + _m)
        _slot += 2
POSARR = np.empty(NCOL, np.int64)
for _i, _j in enumerate(ORDER):
    POSARR[_j] = _i


def _ctil():
    """ctil(l,m) * P_l^m(x) * ang(m, phi) = output column, with the
    reference's 1/sqrt(2) for m=0 folded in. m>=1 entries are negated
    to absorb the -sin from the range-reduced LUT trig."""
    c = {}
    for l in range(LMAX + 1):
        c[(l, 0)] = math.sqrt((2 * l + 1) / (4 * math.pi))
        for m in range(1, l + 1):
            c[(l, m)] = -((-1.0) ** m) * math.sqrt(2.0) * math.sqrt(
                (2 * l + 1) / (4 * math.pi)
                * math.factorial(l - m) / math.factorial(l + m)
            )
    return c


def _lrec_ab(l, m, C):
    """P~(l,m) = a*x*P~(l-1,m) + b*P~(l-2,m)."""
    alpha = (2 * l - 1) / (l - m)
    beta = -(l + m - 1) / (l - m)
    a = alpha * C[(l, m)] / C[(l - 1, m)]
    b = beta * C[(l, m)] / C[(l - 2, m)]
    return a, b


def build_nc(fds=None):
    if fds is None:
        fds = FDS
    lpp = sum(fds)
    assert lpp == LPP
    C = _ctil()
    nc = bass.Bass()
    ct = nc.declare_dram_parameter("cos_theta", [PADN], F32, isOutput=False)
    ph = nc.declare_dram_parameter("phi", [PADN], F32, isOutput=False)
    out = nc.declare_dram_parameter("out", [PADN * NCOL], F16, isOutput=True)

    ctv = ct[:].rearrange("(p f) -> p f", p=P)
    phv = ph[:].rearrange("(p f) -> p f", p=P)
    outv = out[:].rearrange("(p f) -> p f", p=P)

    with TileContext(nc) as tc:
        with (
            tc.tile_pool(name="res", bufs=1) as res_pool,
            tc.tile_pool(name="work", bufs=2) as work_pool,
            tc.tile_pool(name="obuf", bufs=2) as o_pool,
        ):
            V = nc.vector
            G = nc.gpsimd
            A = nc.scalar

            xt = res_pool.tile([P, lpp], F32)
            pt = res_pool.tile([P, lpp], F32)
            cbias = res_pool.tile([P, 2], F32)
            G.memset(cbias[:, 0:1], -PI_LO)
            G.memset(cbias[:, 1:2], -PI_LO / 2)
            bias_negpi = cbias[:, 0:1]
            bias_neghalfpi = cbias[:, 1:2]

            off = 0
            for fd in fds:
                sl = slice(off, off + fd)
                obase = off * NCOL
                off += fd
                nc.sync.dma_start(out=xt[:, sl], in_=ctv[:, sl])
                nc.sync.dma_start(out=pt[:, sl], in_=phv[:, sl])
                x = xt[:, sl]
                f = pt[:, sl]

                # f32 work: 34 fd-slices, all pairs flat/adjacent.
                #  0 x2 | 1 s | 2,3 s2 pair | 4..11 diag ring (4 pairs)
                #  12,13 twoC1 pair | 14,15 trig temp u | 16..31 trig
                #  (SIN|COS pairs, m at 16+2(m-1)) | 32 b | 33 b2
                w = work_pool.tile([P, fd * 34], F32)
                # fp16 work: 0,1 xx pair | 2,3 T pair | 4 T0 (m=0)
                w6 = work_pool.tile([P, fd * 5], F16)

                def W(i):
                    return w[:, i * fd:(i + 1) * fd]

                def WF(i):
                    return w[:, i * fd:(i + 2) * fd]

                def H(i):
                    return w6[:, i * fd:(i + 1) * fd]

                def HF(i):
                    return w6[:, i * fd:(i + 2) * fd]

                XX = H(0)
                XXP = HF(0)
                TP = HF(2)
                T0 = H(4)

                def DPf(m):
                    return WF(4 + 2 * (m & 3))

                def TRIGF(m):
                    return WF(16 + 2 * (m - 1))

                O = o_pool.tile([P, fd * NCOL], F16)

                def om0(l):
                    return O[:, l * fd:(l + 1) * fd]

                def opr(l, m):
                    b0 = PAIRBASE[(l, m)]
                    return O[:, b0 * fd:(b0 + 2) * fd]

                # ---- ACT seeds; trig group first (gates Pool) ----
                A.activation(W(16), f, AF.Sin, bias=bias_negpi)   # SIN1
                A.activation(W(32), f, AF.Sin, scale=0.5, bias=bias_neghalfpi)
                A.activation(W(33), W(32), AF.Square)             # b^2
                A.activation(W(12), W(33), AF.Copy, scale=4.0, bias=-2.0)
                A.activation(W(13), W(33), AF.Copy, scale=4.0, bias=-2.0)
                A.activation(W(17), W(33), AF.Copy, scale=-2.0, bias=1.0)
                A.activation(W(0), x, AF.Square)                      # x^2
                A.activation(W(1), W(0), AF.Sqrt, bias=1.0, scale=-1.0)  # s
                A.activation(W(2), W(0), AF.Copy, scale=-1.0, bias=1.0)  # s2
                A.activation(W(3), W(0), AF.Copy, scale=-1.0, bias=1.0)
                c11, c22 = C[(1, 1)], C[(2, 2)]
                A.activation(W(6), W(1), AF.Copy, scale=-c11)  # D1 pair
                A.activation(W(7), W(1), AF.Copy, scale=-c11)
                A.activation(W(8), W(0), AF.Copy, scale=-3 * c22, bias=3 * c22)
                A.activation(W(9), W(0), AF.Copy, scale=-3 * c22, bias=3 * c22)
                A.activation(H(0), x, AF.Copy)                 # xx fp16 pair
                A.activation(H(1), x, AF.Copy)

                # ---- DVE-only compute: concurrent Pool work steals the
                # shared DVE/GPSIMD SBUF ports (~2.8x DVE slowdown
                # measured), so Pool runs nothing per-chunk. ----
                # f32 trig Chebyshev on SIN|COS pairs
                V.tensor_tensor(TRIGF(2), WF(12), TRIGF(1), OP.mult)
                V.tensor_scalar(W(19), W(19), 1.0, None, OP.add)  # COS2 += 1
                for m in range(3, 9):
                    V.tensor_tensor(WF(14), WF(12), TRIGF(m - 1), OP.mult)
                    V.tensor_tensor(TRIGF(m), WF(14), TRIGF(m - 2), OP.subtract)

                # ---- m = 0 chain (fp16 state lives in the columns) ----
                V.tensor_scalar(om0(0), x, 0.0, C[(0, 0)], OP.mult, OP.add)
                V.tensor_scalar(om0(1), x, C[(1, 0)], None, OP.mult)
                a, bb = _lrec_ab(2, 0, C)
                V.scalar_tensor_tensor(T0, om0(1), a, XX, OP.mult, OP.mult)
                V.tensor_scalar(om0(2), T0, bb * C[(0, 0)], None, OP.add)
                for l in range(3, 9):
                    a, bb = _lrec_ab(l, 0, C)
                    V.scalar_tensor_tensor(T0, om0(l - 1), a, XX, OP.mult, OP.mult)
                    V.scalar_tensor_tensor(
                        om0(l), om0(l - 2), bb, T0, OP.mult, OP.add
                    )

                # ---- per-m: diag (STT, 4-pair ring -- colmult must
                # consume slot m before diag m+4 overwrites it, hence
                # interleaved), column seed (TT), fp16 chains ----
                for m in range(1, 9):
                    if m >= 3:
                        Am = (2 * m - 1) * (2 * m - 3) * C[(m, m)] / C[(m - 2, m - 2)]
                        V.scalar_tensor_tensor(
                            DPf(m), DPf(m - 2), Am, WF(2), OP.mult, OP.mult
                        )
                    V.tensor_tensor(opr(m, m), DPf(m), TRIGF(m), OP.mult)
                    if m > 7:
                        continue
                    Em = (2 * m + 1) * C[(m + 1, m)] / C[(m, m)]
                    V.scalar_tensor_tensor(
                        opr(m + 1, m), opr(m, m), Em, XXP, OP.mult, OP.mult
                    )
                    for l in range(m + 2, 9):
                        a, bb = _lrec_ab(l, m, C)
                        V.scalar_tensor_tensor(
                            TP, opr(l - 1, m), a, XXP, OP.mult, OP.mult
                        )
                        V.scalar_tensor_tensor(
                            opr(l, m), opr(l - 2, m), bb, TP, OP.mult, OP.add
                        )

                nc.sync.dma_start(
                    out=outv[:, obase:obase + fd * NCOL], in_=O[:, :]
                )
    _legalize_waits(nc)
    return nc


def _legalize_waits(nc):
    """TPB compute ISA structs encode a single sync-wait slot; Tile can
    emit 2+ waits on one instruction (walrus then fails with 'Too many
    sync wait commands'). Hoist all but one wait onto NoOps in front."""
    f = nc.m.functions[0]
    for b in f.blocks:
        insts = b.instructions
        idx = 0
        while idx < len(insts):
            i = insts[idx]
            si = i.sync_info
            if si is not None and len(si.on_wait) > 1:
                waits = list(si.on_wait)
                for wextra in waits[:-1]:
                    nop = mybir.InstEventSemaphore(
                        name=nc.get_next_instruction_name(), ins=[], outs=[]
                    )
                    nop.engine = i.engine
                    nop.sync_info = mybir.SyncInfo(
                        on_wait=[wextra], on_update=[]
                    )
                    nc.register_instruction(nop)
                    insts.insert(idx, nop)
                    idx += 1
                si.on_wait = [waits[-1]]
            idx += 1


_NC_CACHE = None


def _get_nc():
    global _NC_CACHE
    if _NC_CACHE is None:
        _NC_CACHE = build_nc()
    return _NC_CACHE


def _run(cos_theta, phi, trace=False, **kw):
    cos_theta = np.ascontiguousarray(np.asarray(cos_theta), dtype=np.float32)
    phi = np.ascontiguousarray(np.asarray(phi), dtype=np.float32)
    assert cos_theta.shape == (N_TOTAL,) and phi.shape == (N_TOTAL,)
    in_maps = []
    for i in range(NCORES):
        c = np.zeros(PADN, np.float32)
        p_ = np.zeros(PADN, np.float32)
        c[:PER] = cos_theta[i * PER:(i + 1) * PER]
        p_[:PER] = phi[i * PER:(i + 1) * PER]
        in_maps.append({"cos_theta": c, "phi": p_})
    res = run_bass_kernel_spmd(
        _get_nc(), in_maps, core_ids=list(range(NCORES)), trace=trace, **kw
    )
    outs = []
    for r in res.results:
        o = np.asarray(r["out"]).reshape(P, LPP * NCOL)  # fp16, slot-major
        full = np.empty((P, LPP, NCOL), np.float32)
        offp = 0
        for fd in FDS:
            blk = o[:, offp * NCOL:(offp + fd) * NCOL].reshape(P, NCOL, fd)
            full[:, offp:offp + fd, :] = blk[:, POSARR, :].transpose(0, 2, 1)
            offp += fd
        outs.append(full.reshape(PADN, NCOL)[:PER])
    return np.concatenate(outs, axis=0), res


def kernel(cos_theta, phi):
    out, _ = _run(cos_theta, phi)
    return out


# revision 11
# speedup vs baseline: 2.1231x; 1.2277x over previous
"""Real spherical harmonics Y_lm (l<=8) on 8 TRN2 NeuronCores.

Data-parallel over the 1M points; per core 125k points padded to
128*977. Output is written in fp16, column-PERMUTED (each chain's
(-m,+m) pair adjacent -> all ops stream flat contiguous runs) and
gamma-SCALED: columns hold Q(l,m) = Y(l,m)/gamma(l,m), with gamma
chosen per chain so the l-recurrence loses its x-coefficient:

    Q(l) = x (.) Q(l-1) + c_l * Q(l-2)

so the x-multiply is a plain fp16 tensor_tensor (2x DVE mode, 382ns
vs 693ns for scalar_tensor_tensor), first-l collapses to one TT, and
the diagonal chain folds its constants into its seeds (fp16 TT). The
host unshard un-permutes, descales by gamma, and converts to f32
(total rel err ~1.4e-3 vs the 2e-2 gate).

All compute runs on DVE + ACT: the Pool engine shares SBUF ports with
DVE (concurrent Pool work measured to slow DVE ops ~2.8x), so Pool is
net-negative despite being idle.
"""

import math
import sys

sys.path.insert(0, "/opt/trn_rl_repo")

import numpy as np

import concourse.bass as bass
import concourse.mybir as mybir
from concourse.tile import TileContext
from concourse.bass_utils import run_bass_kernel_spmd

F32 = mybir.dt.float32
F16 = mybir.dt.float16
AF = mybir.ActivationFunctionType
OP = mybir.AluOpType

N_TOTAL = 1_000_000
NCORES = 8
PER = N_TOTAL // NCORES      # 125000 real points per core
P = 128                      # SBUF partitions
LPP = 977                    # points per partition (padded: 125056)
PADN = P * LPP
LMAX = 8
NCOL = (LMAX + 1) ** 2       # 81
FDS = [300, 300, 300, 77]    # free-dim chunk sizes (sum == LPP)

# ACT Sin LUT domain is [-pi, pi]; we feed t - pi, yielding -sin(t).
# The global -1 is folded into ctil(l,m) for m>=1.
PI_LO = float(np.nextafter(np.float32(math.pi), np.float32(0.0)))

# --- output slot permutation: m=0 columns at slots 0..8, then each
# m-chain's (sin|cos) pairs adjacent, ordered by l ---
ORDER = [l * l + l for l in range(LMAX + 1)]
PAIRBASE = {}
_slot = LMAX + 1
for _m in range(1, LMAX + 1):
    for _l in range(_m, LMAX + 1):
        PAIRBASE[(_l, _m)] = _slot
        ORDER.append(_l * _l + _l - _m)
        ORDER.append(_l * _l + _l + _m)
        _slot += 2
POSARR = np.empty(NCOL, np.int64)
for _i, _j in enumerate(ORDER):
    POSARR[_j] = _i


def _ctil():
    """ctil(l,m) * P_l^m(x) * ang(m, phi) = output column, with the
    reference's 1/sqrt(2) for m=0 folded in. m>=1 entries are negated
    to absorb the -sin from the range-reduced LUT trig."""
    c = {}
    for l in range(LMAX + 1):
        c[(l, 0)] = math.sqrt((2 * l + 1) / (4 * math.pi))
        for m in range(1, l + 1):
            c[(l, m)] = -((-1.0) ** m) * math.sqrt(2.0) * math.sqrt(
                (2 * l + 1) / (4 * math.pi)
                * math.factorial(l - m) / math.factorial(l + m)
            )
    return c


def _lrec_ab(l, m, C):
    """P~(l,m) = a*x*P~(l-1,m) + b*P~(l-2,m)."""
    alpha = (2 * l - 1) / (l - m)
    beta = -(l + m - 1) / (l - m)
    a = alpha * C[(l, m)] / C[(l - 1, m)]
    b = beta * C[(l, m)] / C[(l - 2, m)]
    return a, b


def _scales():
    """Per-column gamma (host descale), folded diag seeds, and the
    residual c coefficients for the scaled recurrences."""
    C = _ctil()
    kk = {1: 1.0, 2: 1.0}   # Dt(m) = kk_m * D_m ; Dt(m) = s2 (.) Dt(m-2)
    for m in range(3, LMAX + 1):
        Am = (2 * m - 1) * (2 * m - 3) * C[(m, m)] / C[(m - 2, m - 2)]
        kk[m] = kk[m - 2] / Am
    gamma = {}
    clm = {}
    for m in range(1, LMAX + 1):
        gamma[(m, m)] = 1.0 / kk[m]
        if m <= LMAX - 1:
            Em = (2 * m + 1) * C[(m + 1, m)] / C[(m, m)]
            gamma[(m + 1, m)] = Em * gamma[(m, m)]
            for l in range(m + 2, LMAX + 1):
                a, bb = _lrec_ab(l, m, C)
                gamma[(l, m)] = a * gamma[(l - 1, m)]
                clm[(l, m)] = bb * gamma[(l - 2, m)] / gamma[(l, m)]
    g0 = {0: C[(0, 0)], 1: C[(1, 0)]}
    c0 = {}
    for l in range(2, LMAX + 1):
        a, bb = _lrec_ab(l, 0, C)
        g0[l] = a * g0[l - 1]
        c0[l] = bb * g0[l - 2] / g0[l]
    gam = np.ones(NCOL, np.float32)
    for l in range(LMAX + 1):
        gam[l * l + l] = g0[l]
    for (l, m), g in gamma.items():
        gam[l * l + l - m] = g
        gam[l * l + l + m] = g
    return C, kk, clm, c0, gam


_C, _KK, _CLM, _C0, GAMMA = _scales()


def build_nc(fds=None):
    if fds is None:
        fds = FDS
    lpp = sum(fds)
    assert lpp == LPP
    C, kk, clm, c0 = _C, _KK, _CLM, _C0
    nc = bass.Bass()
    ct = nc.declare_dram_parameter("cos_theta", [PADN], F32, isOutput=False)
    ph = nc.declare_dram_parameter("phi", [PADN], F32, isOutput=False)
    out = nc.declare_dram_parameter("out", [PADN * NCOL], F16, isOutput=True)

    ctv = ct[:].rearrange("(p f) -> p f", p=P)
    phv = ph[:].rearrange("(p f) -> p f", p=P)
    outv = out[:].rearrange("(p f) -> p f", p=P)

    with TileContext(nc) as tc:
        with (
            tc.tile_pool(name="res", bufs=1) as res_pool,
            tc.tile_pool(name="work", bufs=2) as work_pool,
            tc.tile_pool(name="obuf", bufs=2) as o_pool,
        ):
            V = nc.vector
            A = nc.scalar

            xt = res_pool.tile([P, lpp], F32)
            pt = res_pool.tile([P, lpp], F32)
            cbias = res_pool.tile([P, 2], F32)
            nc.gpsimd.memset(cbias[:, 0:1], -PI_LO)
            nc.gpsimd.memset(cbias[:, 1:2], -PI_LO / 2)
            bias_negpi = cbias[:, 0:1]
            bias_neghalfpi = cbias[:, 1:2]

            off = 0
            for fd in fds:
                sl = slice(off, off + fd)
                obase = off * NCOL
                off += fd
                nc.sync.dma_start(out=xt[:, sl], in_=ctv[:, sl])
                nc.sync.dma_start(out=pt[:, sl], in_=phv[:, sl])
                x = xt[:, sl]
                f = pt[:, sl]

                # f32 work (24 fd-slices): 0 x2 | 1 s | 2,3 twoC1 pair |
                #  4,5 trig temp | 6..21 trig SIN|COS pairs (m at
                #  6+2(m-1)) | 22 b | 23 b2
                w = work_pool.tile([P, fd * 24], F32)
                # fp16 work (17 slices): 0,1 xx pair | 2,3 T pair |
                #  4,5 w pair | 6 T0 | 7,8 s2 pair | 9..16 diag ring
                w6 = work_pool.tile([P, fd * 17], F16)

                def W(i):
                    return w[:, i * fd:(i + 1) * fd]

                def WF(i):
                    return w[:, i * fd:(i + 2) * fd]

                def H(i):
                    return w6[:, i * fd:(i + 1) * fd]

                def HF(i):
                    return w6[:, i * fd:(i + 2) * fd]

                XX = H(0)
                XXP = HF(0)
                TP = HF(2)
                WPR = HF(4)
                T0 = H(6)
                S2P = HF(7)

                def DT(m):
                    return HF(9 + 2 * (m & 3))

                def TRIGF(m):
                    return WF(6 + 2 * (m - 1))

                O = o_pool.tile([P, fd * NCOL], F16)

                def om0(l):
                    return O[:, l * fd:(l + 1) * fd]

                def opr(l, m):
                    b0 = PAIRBASE[(l, m)]
                    return O[:, b0 * fd:(b0 + 2) * fd]

                # ---- ACT seeds; trig group first (gates the chain) ----
                A.activation(W(6), f, AF.Sin, bias=bias_negpi)    # SIN1
                A.activation(W(22), f, AF.Sin, scale=0.5, bias=bias_neghalfpi)
                A.activation(W(23), W(22), AF.Square)             # b^2
                A.activation(W(2), W(23), AF.Copy, scale=4.0, bias=-2.0)
                A.activation(W(3), W(23), AF.Copy, scale=4.0, bias=-2.0)
                A.activation(W(7), W(23), AF.Copy, scale=-2.0, bias=1.0)
                A.activation(W(0), x, AF.Square)                      # x^2
                A.activation(W(1), W(0), AF.Sqrt, bias=1.0, scale=-1.0)  # s
                c11, c22 = C[(1, 1)], C[(2, 2)]
                d1 = -c11 * kk[1]
                d2 = 3.0 * c22 * kk[2]
                A.activation(H(11), W(1), AF.Copy, scale=d1)   # Dt1 pair
                A.activation(H(12), W(1), AF.Copy, scale=d1)
                A.activation(H(13), W(0), AF.Copy, scale=-d2, bias=d2)
                A.activation(H(14), W(0), AF.Copy, scale=-d2, bias=d2)
                A.activation(H(7), W(0), AF.Copy, scale=-1.0, bias=1.0)  # s2
                A.activation(H(8), W(0), AF.Copy, scale=-1.0, bias=1.0)
                A.activation(H(0), x, AF.Copy)                 # xx fp16 pair
                A.activation(H(1), x, AF.Copy)

                # ---- trig Chebyshev (f32 pairs) ----
                V.tensor_tensor(TRIGF(2), WF(2), TRIGF(1), OP.mult)
                V.tensor_scalar(W(9), W(9), 1.0, None, OP.add)  # COS2 += 1
                for m in range(3, 9):
                    V.tensor_tensor(WF(4), WF(2), TRIGF(m - 1), OP.mult)
                    V.tensor_tensor(TRIGF(m), WF(4), TRIGF(m - 2), OP.subtract)

                # ---- m = 0 chain: Q0(0)=1, Q0(1)=x,
                #      Q0(l) = x (.) Q0(l-1) + c0_l Q0(l-2) ----
                V.tensor_scalar(om0(0), x, 0.0, 1.0, OP.mult, OP.add)
                V.tensor_scalar(om0(1), x, 1.0, None, OP.mult)
                V.tensor_tensor(T0, XX, om0(1), OP.mult)
                V.tensor_scalar(om0(2), T0, c0[2], None, OP.add)
                for l in range(3, 9):
                    V.tensor_tensor(T0, XX, om0(l - 1), OP.mult)
                    V.scalar_tensor_tensor(
                        om0(l), om0(l - 2), c0[l], T0, OP.mult, OP.add
                    )

                # ---- per-m: folded diag (fp16 TT, 4-pair ring -- the
                # colmult read must precede the m+4 overwrite, hence
                # interleaved), column seed (mixed TT), scaled chains
                for m in range(1, 9):
                    if m >= 3:
                        V.tensor_tensor(DT(m), S2P, DT(m - 2), OP.mult)
                    V.tensor_tensor(opr(m, m), DT(m), TRIGF(m), OP.mult)
                    if m > 7:
                        continue
                    V.tensor_tensor(opr(m + 1, m), XXP, opr(m, m), OP.mult)
                    for l in range(m + 2, 9):
                        V.tensor_scalar(
                            WPR, opr(l - 2, m), clm[(l, m)], None, OP.mult
                        )
                        V.tensor_tensor(TP, XXP, opr(l - 1, m), OP.mult)
                        V.tensor_tensor(opr(l, m), TP, WPR, OP.add)

                nc.sync.dma_start(
                    out=outv[:, obase:obase + fd * NCOL], in_=O[:, :]
                )
    _legalize_waits(nc)
    return nc


def _legalize_waits(nc):
    """TPB compute ISA structs encode a single sync-wait slot; Tile can
    emit 2+ waits on one instruction (walrus then fails with 'Too many
    sync wait commands'). Hoist all but one wait onto NoOps in front."""
    f = nc.m.functions[0]
    for b in f.blocks:
        insts = b.instructions
        idx = 0
        while idx < len(insts):
            i = insts[idx]
            si = i.sync_info
            if si is not None and len(si.on_wait) > 1:
                waits = list(si.on_wait)
                for wextra in waits[:-1]:
                    nop = mybir.InstEventSemaphore(
                        name=nc.get_next_instruction_name(), ins=[], outs=[]
                    )
                    nop.engine = i.engine
                    nop.sync_info = mybir.SyncInfo(
                        on_wait=[wextra], on_update=[]
                    )
                    nc.register_instruction(nop)
                    insts.insert(idx, nop)
                    idx += 1
                si.on_wait = [waits[-1]]
            idx += 1


_NC_CACHE = None


def _get_nc():
    global _NC_CACHE
    if _NC_CACHE is None:
        _NC_CACHE = build_nc()
    return _NC_CACHE


def _run(cos_theta, phi, trace=False, **kw):
    cos_theta = np.ascontiguousarray(np.asarray(cos_theta), dtype=np.float32)
    phi = np.ascontiguousarray(np.asarray(phi), dtype=np.float32)
    assert cos_theta.shape == (N_TOTAL,) and phi.shape == (N_TOTAL,)
    in_maps = []
    for i in range(NCORES):
        c = np.zeros(PADN, np.float32)
        p_ = np.zeros(PADN, np.float32)
        c[:PER] = cos_theta[i * PER:(i + 1) * PER]
        p_[:PER] = phi[i * PER:(i + 1) * PER]
        in_maps.append({"cos_theta": c, "phi": p_})
    res = run_bass_kernel_spmd(
        _get_nc(), in_maps, core_ids=list(range(NCORES)), trace=trace, **kw
    )
    gscale = GAMMA[None, :, None]  # descale in original-column order
    outs = []
    for r in res.results:
        o = np.asarray(r["out"]).reshape(P, LPP * NCOL)  # fp16, slot-major
        full = np.empty((P, LPP, NCOL), np.float32)
        offp = 0
        for fd in FDS:
            blk = o[:, offp * NCOL:(offp + fd) * NCOL].reshape(P, NCOL, fd)
            full[:, offp:offp + fd, :] = (
                blk[:, POSARR, :] * gscale
            ).transpose(0, 2, 1)
            offp += fd
        outs.append(full.reshape(PADN, NCOL)[:PER])
    return np.concatenate(outs, axis=0), res


def kernel(cos_theta, phi):
    out, _ = _run(cos_theta, phi)
    return out


# revision 12
# speedup vs baseline: 2.5570x; 1.2044x over previous
"""Real spherical harmonics Y_lm (l<=8) on 8 TRN2 NeuronCores.

Data-parallel over the 1M points; per core 125k points padded to
128*977. Output is written in fp16, LEVEL-MAJOR: the slots for level
l are [Q0(l), (sin,cos)(l,1), ..., (sin,cos)(l,l)] with the diagonal
pair last, and columns are gamma-SCALED (Q = Y/gamma with gamma
chosen per chain so every l-recurrence reads

    Q(l) = x (.) Q(l-1) + c * Q(l-2)

with unit x-coefficient). The x-multiply for m0 + ALL chains at a
level is then ONE wide fp16 tensor_tensor whose in0 is x broadcast
via a stride-0 access pattern, and the c*Q(l-2) adds fuse into ONE
wide in-place tensor_tensor; the per-chain c-scale copies run on the
otherwise idle ACT engine (activation Copy with scale). The host
unshard un-permutes, descales, and converts to f32 (~1.4e-3 rel err
vs the 2e-2 gate).

All tensor ops run on DVE: the Pool engine shares SBUF ports with DVE
(concurrent Pool work measured to slow DVE ops ~2.8x), so Pool is
net-negative despite being idle.
"""

import math
import sys

sys.path.insert(0, "/opt/trn_rl_repo")

import numpy as np

import concourse.bass as bass
import concourse.mybir as mybir
from concourse.tile import TileContext
from concourse.bass_utils import run_bass_kernel_spmd

F32 = mybir.dt.float32
F16 = mybir.dt.float16
AF = mybir.ActivationFunctionType
OP = mybir.AluOpType

N_TOTAL = 1_000_000
NCORES = 8
PER = N_TOTAL // NCORES      # 125000 real points per core
P = 128                      # SBUF partitions
LPP = 977                    # points per partition (padded: 125056)
PADN = P * LPP
LMAX = 8
NCOL = (LMAX + 1) ** 2       # 81
FDS = [300, 300, 300, 77]    # free-dim chunk sizes (sum == LPP)

# ACT Sin LUT domain is [-pi, pi]; we feed t - pi, yielding -sin(t).
# The global -1 is folded into ctil(l,m) for m>=1.
PI_LO = float(np.nextafter(np.float32(math.pi), np.float32(0.0)))

# --- level-major slot order: base(l) = l^2; Q0(l) at l^2, sin(l,m) at
# l^2+2m-1, cos(l,m) at l^2+2m ---
ORDER = []
for _l in range(LMAX + 1):
    ORDER.append(_l * _l + _l)           # m=0 column
    for _m in range(1, _l + 1):
        ORDER.append(_l * _l + _l - _m)  # sin
        ORDER.append(_l * _l + _l + _m)  # cos
POSARR = np.empty(NCOL, np.int64)
for _i, _j in enumerate(ORDER):
    POSARR[_j] = _i


def _ctil():
    """ctil(l,m) * P_l^m(x) * ang(m, phi) = output column, with the
    reference's 1/sqrt(2) for m=0 folded in. m>=1 entries are negated
    to absorb the -sin from the range-reduced LUT trig."""
    c = {}
    for l in range(LMAX + 1):
        c[(l, 0)] = math.sqrt((2 * l + 1) / (4 * math.pi))
        for m in range(1, l + 1):
            c[(l, m)] = -((-1.0) ** m) * math.sqrt(2.0) * math.sqrt(
                (2 * l + 1) / (4 * math.pi)
                * math.factorial(l - m) / math.factorial(l + m)
            )
    return c


def _lrec_ab(l, m, C):
    """P~(l,m) = a*x*P~(l-1,m) + b*P~(l-2,m)."""
    alpha = (2 * l - 1) / (l - m)
    beta = -(l + m - 1) / (l - m)
    a = alpha * C[(l, m)] / C[(l - 1, m)]
    b = beta * C[(l, m)] / C[(l - 2, m)]
    return a, b


def _scales():
    """Per-column gamma (host descale), folded diag seeds, and the
    residual c coefficients for the scaled recurrences."""
    C = _ctil()
    kk = {1: 1.0, 2: 1.0}   # Dt(m) = kk_m * D_m ; Dt(m) = s2 (.) Dt(m-2)
    for m in range(3, LMAX + 1):
        Am = (2 * m - 1) * (2 * m - 3) * C[(m, m)] / C[(m - 2, m - 2)]
        kk[m] = kk[m - 2] / Am
    gamma = {}
    clm = {}
    for m in range(1, LMAX + 1):
        gamma[(m, m)] = 1.0 / kk[m]
        if m <= LMAX - 1:
            Em = (2 * m + 1) * C[(m + 1, m)] / C[(m, m)]
            gamma[(m + 1, m)] = Em * gamma[(m, m)]
            for l in range(m + 2, LMAX + 1):
                a, bb = _lrec_ab(l, m, C)
                gamma[(l, m)] = a * gamma[(l - 1, m)]
                clm[(l, m)] = bb * gamma[(l - 2, m)] / gamma[(l, m)]
    g0 = {0: C[(0, 0)], 1: C[(1, 0)]}
    c0 = {}
    for l in range(2, LMAX + 1):
        a, bb = _lrec_ab(l, 0, C)
        g0[l] = a * g0[l - 1]
        c0[l] = bb * g0[l - 2] / g0[l]
    gam = np.ones(NCOL, np.float32)
    for l in range(LMAX + 1):
        gam[l * l + l] = g0[l]
    for (l, m), g in gamma.items():
        gam[l * l + l - m] = g
        gam[l * l + l + m] = g
    return C, kk, clm, c0, gam


_C, _KK, _CLM, _C0, GAMMA = _scales()


def build_nc(fds=None):
    if fds is None:
        fds = FDS
    lpp = sum(fds)
    assert lpp == LPP
    C, kk, clm, c0 = _C, _KK, _CLM, _C0
    nc = bass.Bass()
    ct = nc.declare_dram_parameter("cos_theta", [PADN], F32, isOutput=False)
    ph = nc.declare_dram_parameter("phi", [PADN], F32, isOutput=False)
    out = nc.declare_dram_parameter("out", [PADN * NCOL], F16, isOutput=True)

    ctv = ct[:].rearrange("(p f) -> p f", p=P)
    phv = ph[:].rearrange("(p f) -> p f", p=P)
    outv = out[:].rearrange("(p f) -> p f", p=P)

    with TileContext(nc) as tc:
        with (
            tc.tile_pool(name="res", bufs=1) as res_pool,
            tc.tile_pool(name="work", bufs=2) as work_pool,
            tc.tile_pool(name="obuf", bufs=2) as o_pool,
        ):
            V = nc.vector
            A = nc.scalar

            xt = res_pool.tile([P, lpp], F32)
            pt = res_pool.tile([P, lpp], F32)
            cbias = res_pool.tile([P, 2], F32)
            nc.gpsimd.memset(cbias[:, 0:1], -PI_LO)
            nc.gpsimd.memset(cbias[:, 1:2], -PI_LO / 2)
            bias_negpi = cbias[:, 0:1]
            bias_neghalfpi = cbias[:, 1:2]

            off = 0
            for fd in fds:
                sl = slice(off, off + fd)
                obase = off * NCOL
                off += fd
                nc.sync.dma_start(out=xt[:, sl], in_=ctv[:, sl])
                nc.sync.dma_start(out=pt[:, sl], in_=phv[:, sl])
                x = xt[:, sl]
                f = pt[:, sl]

                # f32 work (24 fd-slices): 0 x2 | 1 s | 2,3 twoC1 pair |
                #  4,5 trig temp | 6..21 trig SIN|COS pairs (m at
                #  6+2(m-1)) | 22 b | 23 b2
                w = work_pool.tile([P, fd * 24], F32)
                # fp16 work (25 slices): 0,1 xx pair | 2..14 W arena
                #  (c-scaled adds, max width 13) | 15,16 s2 pair |
                #  17..24 diag ring (4 pairs)
                w6 = work_pool.tile([P, fd * 25], F16)

                def W(i):
                    return w[:, i * fd:(i + 1) * fd]

                def WF(i):
                    return w[:, i * fd:(i + 2) * fd]

                def H(i):
                    return w6[:, i * fd:(i + 1) * fd]

                def HF(i):
                    return w6[:, i * fd:(i + 2) * fd]

                XX = H(0)
                XXP = HF(0)

                def WA(k, n):   # W arena slots k..k+n
                    return w6[:, (2 + k) * fd:(2 + k + n) * fd]

                S2P = HF(15)

                def DT(m):
                    return HF(17 + 2 * (m & 3))

                def TRIGF(m):
                    return WF(6 + 2 * (m - 1))

                O = o_pool.tile([P, fd * NCOL], F16)

                def oblk(l, n):   # level-l block prefix, n slots
                    return O[:, l * l * fd:(l * l + n) * fd]

                def om0(l):
                    return oblk(l, 1)

                def opr(l, m):
                    b0 = l * l + 2 * m - 1
                    return O[:, b0 * fd:(b0 + 2) * fd]

                def bcast(ap, n):
                    return ap.rearrange("p (k f) -> p k f", k=1).broadcast_to(
                        [P, n, fd]
                    )

                def r3(ap):
                    return ap.rearrange("p (k f) -> p k f", f=fd)

                # ---- ACT seeds; trig group first (gates the chain) ----
                A.activation(W(6), f, AF.Sin, bias=bias_negpi)    # SIN1
                A.activation(W(22), f, AF.Sin, scale=0.5, bias=bias_neghalfpi)
                A.activation(W(23), W(22), AF.Square)             # b^2
                A.activation(W(2), W(23), AF.Copy, scale=4.0, bias=-2.0)
                A.activation(W(3), W(23), AF.Copy, scale=4.0, bias=-2.0)
                A.activation(W(7), W(23), AF.Copy, scale=-2.0, bias=1.0)
                A.activation(W(0), x, AF.Square)                      # x^2
                A.activation(W(1), W(0), AF.Sqrt, bias=1.0, scale=-1.0)  # s
                c11, c22 = C[(1, 1)], C[(2, 2)]
                d1 = -c11 * kk[1]
                d2 = 3.0 * c22 * kk[2]
                A.activation(H(19), W(1), AF.Copy, scale=d1)   # Dt1 pair
                A.activation(H(20), W(1), AF.Copy, scale=d1)
                A.activation(H(21), W(0), AF.Copy, scale=-d2, bias=d2)
                A.activation(H(22), W(0), AF.Copy, scale=-d2, bias=d2)
                A.activation(H(15), W(0), AF.Copy, scale=-1.0, bias=1.0)  # s2
                A.activation(H(16), W(0), AF.Copy, scale=-1.0, bias=1.0)
                A.activation(H(0), x, AF.Copy)                 # xx fp16 pair
                A.activation(H(1), x, AF.Copy)

                # ---- trig Chebyshev (f32 pairs) ----
                V.tensor_tensor(TRIGF(2), WF(2), TRIGF(1), OP.mult)
                V.tensor_scalar(W(9), W(9), 1.0, None, OP.add)  # COS2 += 1
                for m in range(3, 9):
                    V.tensor_tensor(WF(4), WF(2), TRIGF(m - 1), OP.mult)
                    V.tensor_tensor(TRIGF(m), WF(4), TRIGF(m - 2), OP.subtract)

                # ---- levels 0..2 (seeds and degenerate recurrences) ----
                V.tensor_scalar(om0(0), x, 0.0, 1.0, OP.mult, OP.add)
                V.tensor_scalar(om0(1), x, 1.0, None, OP.mult)
                V.tensor_tensor(opr(1, 1), DT(1), TRIGF(1), OP.mult)
                V.tensor_tensor(om0(2), XX, om0(1), OP.mult)
                V.tensor_scalar(om0(2), om0(2), c0[2], None, OP.add)
                V.tensor_tensor(opr(2, 1), XXP, opr(1, 1), OP.mult)
                V.tensor_tensor(opr(2, 2), DT(2), TRIGF(2), OP.mult)

                # ---- levels 3..8: wide x-multiply over [m0 + lrec
                # chains], ACT c-scale preps, wide in-place add, then
                # first-l, diag, colmult ----
                for l in range(3, 9):
                    wd = 1 + 2 * (l - 2)   # m0 + chains m=1..l-2
                    V.tensor_tensor(
                        r3(oblk(l, wd)), bcast(XX, wd), r3(oblk(l - 1, wd)),
                        OP.mult,
                    )
                    # c-scale preps on ACT: W arena mirrors the FULL
                    # level-(l-2) block (its last pair is chain m=l-2's
                    # Q(l-2), i.e. the diagonal pair)
                    A.activation(WA(0, 1), om0(l - 2), AF.Copy, scale=c0[l])
                    for m in range(1, l - 1):
                        A.activation(
                            WA(2 * m - 1, 2), opr(l - 2, m), AF.Copy,
                            scale=clm[(l, m)],
                        )
                    V.tensor_tensor(
                        oblk(l, wd), oblk(l, wd), WA(0, wd), OP.add
                    )
                    V.tensor_tensor(opr(l, l - 1), XXP, opr(l - 1, l - 1), OP.mult)
                    V.tensor_tensor(DT(l), S2P, DT(l - 2), OP.mult)
                    V.tensor_tensor(opr(l, l), DT(l), TRIGF(l), OP.mult)

                nc.sync.dma_start(
                    out=outv[:, obase:obase + fd * NCOL], in_=O[:, :]
                )
    _legalize_waits(nc)
    return nc


def _legalize_waits(nc):
    """TPB compute ISA structs encode a single sync-wait slot; Tile can
    emit 2+ waits on one instruction (walrus then fails with 'Too many
    sync wait commands'). Hoist all but one wait onto NoOps in front."""
    f = nc.m.functions[0]
    for b in f.blocks:
        insts = b.instructions
        idx = 0
        while idx < len(insts):
            i = insts[idx]
            si = i.sync_info
            if si is not None and len(si.on_wait) > 1:
                waits = list(si.on_wait)
                for wextra in waits[:-1]:
                    nop = mybir.InstEventSemaphore(
                        name=nc.get_next_instruction_name(), ins=[], outs=[]
                    )
                    nop.engine = i.engine
                    nop.sync_info = mybir.SyncInfo(
                        on_wait=[wextra], on_update=[]
                    )
                    nc.register_instruction(nop)
                    insts.insert(idx, nop)
                    idx += 1
                si.on_wait = [waits[-1]]
            idx += 1


_NC_CACHE = None


def _get_nc():
    global _NC_CACHE
    if _NC_CACHE is None:
        _NC_CACHE = build_nc()
    return _NC_CACHE


def _run(cos_theta, phi, trace=False, **kw):
    cos_theta = np.ascontiguousarray(np.asarray(cos_theta), dtype=np.float32)
    phi = np.ascontiguousarray(np.asarray(phi), dtype=np.float32)
    assert cos_theta.shape == (N_TOTAL,) and phi.shape == (N_TOTAL,)
    in_maps = []
    for i in range(NCORES):
        c = np.zeros(PADN, np.float32)
        p_ = np.zeros(PADN, np.float32)
        c[:PER] = cos_theta[i * PER:(i + 1) * PER]
        p_[:PER] = phi[i * PER:(i + 1) * PER]
        in_maps.append({"cos_theta": c, "phi": p_})
    res = run_bass_kernel_spmd(
        _get_nc(), in_maps, core_ids=list(range(NCORES)), trace=trace, **kw
    )
    gscale = GAMMA[None, :, None]  # descale in original-column order
    outs = []
    for r in res.results:
        o = np.asarray(r["out"]).reshape(P, LPP * NCOL)  # fp16, slot-major
        full = np.empty((P, LPP, NCOL), np.float32)
        offp = 0
        for fd in FDS:
            blk = o[:, offp * NCOL:(offp + fd) * NCOL].reshape(P, NCOL, fd)
            full[:, offp:offp + fd, :] = (
                blk[:, POSARR, :] * gscale
            ).transpose(0, 2, 1)
            offp += fd
        outs.append(full.reshape(PADN, NCOL)[:PER])
    return np.concatenate(outs, axis=0), res


def kernel(cos_theta, phi):
    out, _ = _run(cos_theta, phi)
    return out


# revision 17
# speedup vs baseline: 2.6361x; 1.0309x over previous
"""Real spherical harmonics Y_lm (l<=8) on 8 TRN2 NeuronCores.

Data-parallel over the 1M points; per core 125k points padded to
128*977. Output is written in fp16, LEVEL-MAJOR: the slots for level
l are [Q0(l), (sin,cos)(l,1), ..., (sin,cos)(l,l)] with the diagonal
pair last, and columns are gamma-SCALED (Q = Y/gamma with gamma
chosen per chain so every l-recurrence reads

    Q(l) = x (.) Q(l-1) + c * Q(l-2)

with unit x-coefficient). The x-multiply for m0 + ALL chains at a
level is then ONE wide fp16 tensor_tensor whose in0 is x broadcast
via a stride-0 access pattern, and the c*Q(l-2) adds fuse into ONE
wide in-place tensor_tensor; the per-chain c-scale copies run on the
otherwise idle ACT engine (activation Copy with scale). The host
unshard un-permutes, descales, and converts to f32 (~1.4e-3 rel err
vs the 2e-2 gate).

All tensor ops run on DVE: the Pool engine shares SBUF ports with DVE
(concurrent Pool work measured to slow DVE ops ~2.8x), so Pool is
net-negative despite being idle.
"""

import math
import sys

sys.path.insert(0, "/opt/trn_rl_repo")

import numpy as np

import concourse.bass as bass
import concourse.mybir as mybir
from concourse.tile import TileContext
from concourse.bass_utils import run_bass_kernel_spmd

F32 = mybir.dt.float32
F16 = mybir.dt.float16
AF = mybir.ActivationFunctionType
OP = mybir.AluOpType

N_TOTAL = 1_000_000
NCORES = 8
PER = N_TOTAL // NCORES      # 125000 real points per core
P = 128                      # SBUF partitions
LPP = 977                    # points per partition (padded: 125056)
PADN = P * LPP
LMAX = 8
NCOL = (LMAX + 1) ** 2       # 81
FDS = [300, 300, 300, 77]    # free-dim chunk sizes (sum == LPP)

# ACT Sin LUT domain is [-pi, pi]; we feed t - pi, yielding -sin(t).
# The global -1 is folded into ctil(l,m) for m>=1.
PI_LO = float(np.nextafter(np.float32(math.pi), np.float32(0.0)))

# --- level-major slot order: base(l) = l^2; Q0(l) at l^2, sin(l,m) at
# l^2+2m-1, cos(l,m) at l^2+2m ---
ORDER = []
for _l in range(LMAX + 1):
    ORDER.append(_l * _l + _l)           # m=0 column
    for _m in range(1, _l + 1):
        ORDER.append(_l * _l + _l - _m)  # sin
        ORDER.append(_l * _l + _l + _m)  # cos
POSARR = np.empty(NCOL, np.int64)
for _i, _j in enumerate(ORDER):
    POSARR[_j] = _i


def _ctil():
    """ctil(l,m) * P_l^m(x) * ang(m, phi) = output column, with the
    reference's 1/sqrt(2) for m=0 folded in. m>=1 entries are negated
    to absorb the -sin from the range-reduced LUT trig."""
    c = {}
    for l in range(LMAX + 1):
        c[(l, 0)] = math.sqrt((2 * l + 1) / (4 * math.pi))
        for m in range(1, l + 1):
            c[(l, m)] = -((-1.0) ** m) * math.sqrt(2.0) * math.sqrt(
                (2 * l + 1) / (4 * math.pi)
                * math.factorial(l - m) / math.factorial(l + m)
            )
    return c


def _lrec_ab(l, m, C):
    """P~(l,m) = a*x*P~(l-1,m) + b*P~(l-2,m)."""
    alpha = (2 * l - 1) / (l - m)
    beta = -(l + m - 1) / (l - m)
    a = alpha * C[(l, m)] / C[(l - 1, m)]
    b = beta * C[(l, m)] / C[(l - 2, m)]
    return a, b


def _scales():
    """Per-column gamma (host descale), folded diag seeds, and the
    residual c coefficients for the scaled recurrences."""
    C = _ctil()
    kk = {1: 1.0, 2: 1.0}   # Dt(m) = kk_m * D_m ; Dt(m) = s2 (.) Dt(m-2)
    for m in range(3, LMAX + 1):
        Am = (2 * m - 1) * (2 * m - 3) * C[(m, m)] / C[(m - 2, m - 2)]
        kk[m] = kk[m - 2] / Am
    gamma = {}
    clm = {}
    for m in range(1, LMAX + 1):
        gamma[(m, m)] = 1.0 / kk[m]
        if m <= LMAX - 1:
            Em = (2 * m + 1) * C[(m + 1, m)] / C[(m, m)]
            gamma[(m + 1, m)] = Em * gamma[(m, m)]
            for l in range(m + 2, LMAX + 1):
                a, bb = _lrec_ab(l, m, C)
                gamma[(l, m)] = a * gamma[(l - 1, m)]
                clm[(l, m)] = bb * gamma[(l - 2, m)] / gamma[(l, m)]
    g0 = {0: C[(0, 0)], 1: C[(1, 0)]}
    c0 = {}
    for l in range(2, LMAX + 1):
        a, bb = _lrec_ab(l, 0, C)
        g0[l] = a * g0[l - 1]
        c0[l] = bb * g0[l - 2] / g0[l]
    gam = np.ones(NCOL, np.float32)
    for l in range(LMAX + 1):
        gam[l * l + l] = g0[l]
    for (l, m), g in gamma.items():
        gam[l * l + l - m] = g
        gam[l * l + l + m] = g
    return C, kk, clm, c0, gam


_C, _KK, _CLM, _C0, GAMMA = _scales()


def build_nc(fds=None):
    if fds is None:
        fds = FDS
    lpp = sum(fds)
    assert lpp == LPP
    C, kk, clm, c0 = _C, _KK, _CLM, _C0
    nc = bass.Bass()
    ct = nc.declare_dram_parameter("cos_theta", [PADN], F32, isOutput=False)
    ph = nc.declare_dram_parameter("phi", [PADN], F32, isOutput=False)
    out = nc.declare_dram_parameter("out", [PADN * NCOL], F16, isOutput=True)

    ctv = ct[:].rearrange("(p f) -> p f", p=P)
    phv = ph[:].rearrange("(p f) -> p f", p=P)
    outv = out[:].rearrange("(p f) -> p f", p=P)

    with TileContext(nc) as tc:
        with (
            tc.tile_pool(name="res", bufs=1) as res_pool,
            tc.tile_pool(name="work", bufs=2) as work_pool,
            tc.tile_pool(name="obuf", bufs=2) as o_pool,
        ):
            V = nc.vector
            A = nc.scalar

            xt = res_pool.tile([P, lpp], F32)
            pt = res_pool.tile([P, lpp], F32)
            cbias = res_pool.tile([P, 4], F32)
            nc.gpsimd.memset(cbias[:, 0:1], -PI_LO)
            nc.gpsimd.memset(cbias[:, 1:2], -PI_LO / 2)
            bias_negpi = cbias[:, 0:1]
            bias_neghalfpi = cbias[:, 1:2]
            # warm the ACT LUT sets (Sin, Sqrt/Square) during input DMA
            nc.scalar.activation(cbias[:, 2:3], cbias[:, 0:1], AF.Sin)
            nc.scalar.activation(cbias[:, 3:4], cbias[:, 0:1], AF.Sqrt)

            off = 0
            for fd in fds:
                sl = slice(off, off + fd)
                obase = off * NCOL
                off += fd
                nc.sync.dma_start(out=xt[:, sl], in_=ctv[:, sl])
                nc.sync.dma_start(out=pt[:, sl], in_=phv[:, sl])
                x = xt[:, sl]
                f = pt[:, sl]

                # f32 work (24 fd-slices): 0 x2 | 1 s | 2,3 twoC1 pair |
                #  4,5 trig temp | 6..21 trig SIN|COS pairs (m at
                #  6+2(m-1)) | 22 b | 23 b2
                w = work_pool.tile([P, fd * 24], F32)
                # fp16 work (25 slices): 0,1 xx pair | 2..14 W arena
                #  (c-scaled adds, max width 13) | 15,16 s2 pair |
                #  17..24 diag ring (4 pairs)
                w6 = work_pool.tile([P, fd * 25], F16)

                def W(i):
                    return w[:, i * fd:(i + 1) * fd]

                def WF(i):
                    return w[:, i * fd:(i + 2) * fd]

                def H(i):
                    return w6[:, i * fd:(i + 1) * fd]

                def HF(i):
                    return w6[:, i * fd:(i + 2) * fd]

                XX = H(0)
                XXP = HF(0)

                def WA(k, n):   # W arena slots k..k+n
                    return w6[:, (2 + k) * fd:(2 + k + n) * fd]

                S2P = HF(15)

                def DT(m):
                    return HF(17 + 2 * (m & 3))

                def TRIGF(m):
                    return WF(6 + 2 * (m - 1))

                O = o_pool.tile([P, fd * NCOL], F16)

                def oblk(l, n):   # level-l block prefix, n slots
                    return O[:, l * l * fd:(l * l + n) * fd]

                def om0(l):
                    return oblk(l, 1)

                def opr(l, m):
                    b0 = l * l + 2 * m - 1
                    return O[:, b0 * fd:(b0 + 2) * fd]

                def bcast(ap, n):
                    return ap.rearrange("p (k f) -> p k f", k=1).broadcast_to(
                        [P, n, fd]
                    )

                def r3(ap):
                    return ap.rearrange("p (k f) -> p k f", f=fd)

                # ---- DVE self-starters (need only x, which lands
                # first): xx fp16 pair, m0 levels 0-1 ----
                V.tensor_scalar(H(0), x, 1.0, None, OP.mult)
                V.tensor_scalar(H(1), x, 1.0, None, OP.mult)
                V.tensor_scalar(om0(0), x, 0.0, 1.0, OP.mult, OP.add)
                V.tensor_scalar(om0(1), x, 1.0, None, OP.mult)
                V.tensor_tensor(om0(2), XX, om0(1), OP.mult)
                V.tensor_scalar(om0(2), om0(2), c0[2], None, OP.add)

                # ---- ACT seeds; trig group first (gates the chain) ----
                A.activation(W(6), f, AF.Sin, bias=bias_negpi)    # SIN1
                A.activation(W(22), f, AF.Sin, scale=0.5, bias=bias_neghalfpi)
                A.activation(W(23), W(22), AF.Square)             # b^2
                A.activation(W(2), W(23), AF.Copy, scale=4.0, bias=-2.0)
                A.activation(W(3), W(23), AF.Copy, scale=4.0, bias=-2.0)
                A.activation(W(7), W(23), AF.Copy, scale=-2.0, bias=1.0)
                A.activation(W(0), x, AF.Square)                      # x^2
                A.activation(W(1), W(0), AF.Sqrt, bias=1.0, scale=-1.0)  # s
                c11, c22 = C[(1, 1)], C[(2, 2)]
                d1 = -c11 * kk[1]
                d2 = 3.0 * c22 * kk[2]
                A.activation(H(19), W(1), AF.Copy, scale=d1)   # Dt1 pair
                A.activation(H(20), W(1), AF.Copy, scale=d1)
                A.activation(H(21), W(0), AF.Copy, scale=-d2, bias=d2)
                A.activation(H(22), W(0), AF.Copy, scale=-d2, bias=d2)
                A.activation(H(15), W(0), AF.Copy, scale=-1.0, bias=1.0)  # s2
                A.activation(H(16), W(0), AF.Copy, scale=-1.0, bias=1.0)

                # ---- trig Chebyshev (f32 pairs) ----
                V.tensor_tensor(TRIGF(2), WF(2), TRIGF(1), OP.mult)
                V.tensor_scalar(W(9), W(9), 1.0, None, OP.add)  # COS2 += 1
                for m in range(3, 9):
                    V.tensor_tensor(WF(4), WF(2), TRIGF(m - 1), OP.mult)
                    V.tensor_tensor(TRIGF(m), WF(4), TRIGF(m - 2), OP.subtract)

                # ---- levels 1..2 column seeds ----
                V.tensor_tensor(opr(1, 1), DT(1), TRIGF(1), OP.mult)
                V.tensor_tensor(opr(2, 1), XXP, opr(1, 1), OP.mult)
                V.tensor_tensor(opr(2, 2), DT(2), TRIGF(2), OP.mult)

                # ---- levels 3..8: wide x-multiply over [m0 + lrec
                # chains], ACT c-scale preps, wide in-place add, then
                # first-l, diag, colmult ----
                for l in range(3, 9):
                    wd = 1 + 2 * (l - 2)   # m0 + chains m=1..l-2
                    V.tensor_tensor(
                        r3(oblk(l, wd)), bcast(XX, wd), r3(oblk(l - 1, wd)),
                        OP.mult,
                    )
                    # c-scale preps on ACT: W arena mirrors the FULL
                    # level-(l-2) block (its last pair is chain m=l-2's
                    # Q(l-2), i.e. the diagonal pair)
                    A.activation(WA(0, 1), om0(l - 2), AF.Copy, scale=c0[l])
                    for m in range(1, l - 1):
                        A.activation(
                            WA(2 * m - 1, 2), opr(l - 2, m), AF.Copy,
                            scale=clm[(l, m)],
                        )
                    V.tensor_tensor(
                        oblk(l, wd), oblk(l, wd), WA(0, wd), OP.add
                    )
                    V.tensor_tensor(opr(l, l - 1), XXP, opr(l - 1, l - 1), OP.mult)
                    V.tensor_tensor(DT(l), S2P, DT(l - 2), OP.mult)
                    V.tensor_tensor(opr(l, l), DT(l), TRIGF(l), OP.mult)
                    # stream finished level groups out early; the final
                    # DMA after the last compute is then only level 8
                    if l in (5, 7, 8):
                        s0 = {5: 0, 7: 36, 8: 64}[l] * fd
                        s1 = (l + 1) * (l + 1) * fd
                        nc.sync.dma_start(
                            out=outv[:, obase + s0:obase + s1],
                            in_=O[:, s0:s1],
                        )
    _legalize_waits(nc)
    return nc


def _legalize_waits(nc):
    """TPB compute ISA structs encode a single sync-wait slot; Tile can
    emit 2+ waits on one instruction (walrus then fails with 'Too many
    sync wait commands'). Hoist all but one wait onto NoOps in front."""
    f = nc.m.functions[0]
    for b in f.blocks:
        insts = b.instructions
        idx = 0
        while idx < len(insts):
            i = insts[idx]
            si = i.sync_info
            if si is not None and len(si.on_wait) > 1:
                waits = list(si.on_wait)
                for wextra in waits[:-1]:
                    nop = mybir.InstEventSemaphore(
                        name=nc.get_next_instruction_name(), ins=[], outs=[]
                    )
                    nop.engine = i.engine
                    nop.sync_info = mybir.SyncInfo(
                        on_wait=[wextra], on_update=[]
                    )
                    nc.register_instruction(nop)
                    insts.insert(idx, nop)
                    idx += 1
                si.on_wait = [waits[-1]]
            idx += 1


_NC_CACHE = None


def _get_nc():
    global _NC_CACHE
    if _NC_CACHE is None:
        _NC_CACHE = build_nc()
    return _NC_CACHE


def _run(cos_theta, phi, trace=False, **kw):
    cos_theta = np.ascontiguousarray(np.asarray(cos_theta), dtype=np.float32)
    phi = np.ascontiguousarray(np.asarray(phi), dtype=np.float32)
    assert cos_theta.shape == (N_TOTAL,) and phi.shape == (N_TOTAL,)
    in_maps = []
    for i in range(NCORES):
        c = np.zeros(PADN, np.float32)
        p_ = np.zeros(PADN, np.float32)
        c[:PER] = cos_theta[i * PER:(i + 1) * PER]
        p_[:PER] = phi[i * PER:(i + 1) * PER]
        in_maps.append({"cos_theta": c, "phi": p_})
    res = run_bass_kernel_spmd(
        _get_nc(), in_maps, core_ids=list(range(NCORES)), trace=trace, **kw
    )
    gscale = GAMMA[None, :, None]  # descale in original-column order
    outs = []
    for r in res.results:
        o = np.asarray(r["out"]).reshape(P, LPP * NCOL)  # fp16, slot-major
        full = np.empty((P, LPP, NCOL), np.float32)
        offp = 0
        for fd in FDS:
            blk = o[:, offp * NCOL:(offp + fd) * NCOL].reshape(P, NCOL, fd)
            full[:, offp:offp + fd, :] = (
                blk[:, POSARR, :] * gscale
            ).transpose(0, 2, 1)
            offp += fd
        outs.append(full.reshape(PADN, NCOL)[:PER])
    return np.concatenate(outs, axis=0), res


def kernel(cos_theta, phi):
    out, _ = _run(cos_theta, phi)
    return out
